# revision 11
# baseline (speedup 1.0000x reference)
"""Co-Attention kernel for Trainium2, 8-core SPMD.

Sharding: spatial (H rows) across 8 cores; 32 rows/core with 1-row halo.
Per-core pipeline (all fused, single launch):
  - load input strips into a guard-padded SBUF layout (258-pitch rows)
  - conv1x1+dwconv3x3 folded: 9 PSUM-accumulated matmuls with shifted APs
    (W3_t[o,c] = W1[o,c] * wdw[o,t]) for each of 5 output units
    (q, k_prev, v_prev, k_next, v_next)
  - q/k: PE transpose -> bf16 [n,c] tiles -> Gram matrices (q@kT, self-Grams
    for L2 norms) accumulated on PE over the core's spatial shard
  - v: v_prev+v_next accumulated into an SBUF-resident strip
  - AllReduce of the tiny Gram/norm stats across the 8 cores
  - on-chip double softmax (block-diagonal channel attention)
  - output = (w_proj @ blockdiag(attn_co)) @ v_sum, one matmul per chunk
"""

import sys

sys.path.insert(0, "/opt/trn_rl_repo")

import numpy as np

import concourse.bacc as bacc
import concourse.bass as bass
import concourse.tile as tile
from concourse import mybir
from concourse.bass_utils import run_bass_kernel_spmd

# problem constants
B, C, H, W = 2, 96, 256, 256
HEADS = 4
CH = C // HEADS
N_CORES = 8
RPC = H // N_CORES          # rows per core (32)
SROWS = RPC + 2             # strip rows incl halo (34)
PITCH = W + 2               # guarded row pitch (258)
LEAD = 2                    # leading guard pad
XLEN = LEAD + SROWS * PITCH + 2  # strip flat length (8776)
NTILES = RPC * 2            # 128-wide transpose tiles per unit per b (64)
VLEN = RPC * PITCH          # v_sum flat length per b (8256)

F32 = mybir.dt.float32
F32R = mybir.dt.float32r
BF16 = mybir.dt.bfloat16

# tap offsets (cross-correlation, matching jax.lax.conv_general_dilated)
TAPS = [(ky - 1) * PITCH + (kx - 1) for ky in range(3) for kx in range(3)]

_CACHE = {}


def rowoff(r):
    return LEAD + r * PITCH


def build_kernel():
    import os as _os
    N_UNITS = int(_os.environ.get("N_UNITS", "5"))
    N_B = int(_os.environ.get("N_B", str(B)))
    N_CHUNK = int(_os.environ.get("N_CHUNK", str(RPC // 2)))
    SKIP_SM = bool(_os.environ.get("SKIP_SM"))
    nc = bacc.Bacc("TRN2", target_bir_lowering=False, debug=False,
                   num_devices=N_CORES)

    xc = nc.declare_dram_parameter("xc", [B, C, SROWS, W], F32R, isOutput=False)
    xp = nc.declare_dram_parameter("xp", [B, C, SROWS, W], F32R, isOutput=False)
    xn = nc.declare_dram_parameter("xn", [B, C, SROWS, W], F32R, isOutput=False)
    w3 = nc.declare_dram_parameter("w3", [C, 45, C], F32R, isOutput=False)
    wpt = nc.declare_dram_parameter("wpt", [C, C], F32, isOutput=False)
    tmp = nc.declare_dram_parameter("tmp", [C, 1], F32, isOutput=False)
    idn = nc.declare_dram_parameter("idn", [C, C], F32, isOutput=False)
    hmk = nc.declare_dram_parameter("hmk", [C, HEADS], F32, isOutput=False)
    bmk = nc.declare_dram_parameter("bmk", [C, C], F32, isOutput=False)
    zz = nc.declare_dram_parameter("zz", [C, 128], F32R, isOutput=False)
    y = nc.declare_dram_parameter("y", [B, C, RPC, W], F32, isOutput=True)

    ar_in = nc.dram_tensor("ar_in", [C, 2 * 195], F32)
    ar_out = nc.dram_tensor("ar_out", [C, 2 * 195], F32, addr_space="Shared")

    xsrc = {0: xc, 1: xp, 2: xn}

    with tile.TileContext(nc) as tc:
        with (
            tc.tile_pool(name="singles", bufs=1) as singles,
            tc.tile_pool(name="xpool", bufs=2) as xpool,
            tc.tile_pool(name="dwsb", bufs=3) as dwsbp,
            tc.tile_pool(name="store", bufs=1) as storep,
            tc.tile_pool(name="kstore", bufs=2) as kstorep,
            tc.tile_pool(name="small", bufs=4) as smallp,
            tc.tile_pool(name="outp", bufs=3) as outp,
            tc.tile_pool(name="psdw", bufs=2, space="PSUM") as psdw,
            tc.tile_pool(name="pstp", bufs=2, space="PSUM") as pstp,
            tc.tile_pool(name="psg", bufs=1, space="PSUM") as psg,
        ):
            # ---- constants ----
            w3_sb = singles.tile([C, 45, C], F32R)
            nc.sync.dma_start(out=w3_sb[:], in_=w3[:, :, :])
            wpt_sb = singles.tile([C, C], F32)
            nc.sync.dma_start(out=wpt_sb[:], in_=wpt[:, :])
            temp_sb = singles.tile([C, 1], F32)
            nc.sync.dma_start(out=temp_sb[:], in_=tmp[:, :])
            ident = singles.tile([C, C], F32)
            nc.sync.dma_start(out=ident[:], in_=idn[:, :])
            identb = singles.tile([C, C], BF16)
            nc.vector.tensor_copy(out=identb[:], in_=ident[:])
            ones1 = singles.tile([1, C], F32)
            nc.vector.memset(ones1[:], 1.0)
            hmask = singles.tile([C, HEADS], F32)
            nc.sync.dma_start(out=hmask[:], in_=hmk[:, :])
            bmask = singles.tile([C, C], F32)
            nc.sync.dma_start(out=bmask[:], in_=bmk[:, :])

            # persistent accumulators
            v_sum = singles.tile([C, B, VLEN], BF16)
            ar_sb = singles.tile([C, B, 195], F32)
            gram_sb = singles.tile([C, B, 5, C], F32)
            arr_sb = singles.tile([C, B, 195], F32)
            mct_sb = singles.tile([C, B, C], BF16)

            qstore = storep.tile([128, NTILES, C], BF16)

            # ---------------- pass 1: conv + dw + grams + v_sum ----------
            for b in range(N_B):
                x_sb = {}
                kT_cur = None
                for u in range(N_UNITS):
                    xi = [0, 1, 1, 2, 2][u]
                    if xi not in x_sb:
                        xt = xpool.tile([C, XLEN], F32R, tag="xstrip")
                        # zero guards: leading, trailing, per-row guard cols
                        # (DMA from a DRAM zeros tensor; memset can't emit
                        # fp32r-typed values)
                        nc.sync.dma_start(out=xt[:, 0:LEAD], in_=zz[:, 0:LEAD])
                        nc.sync.dma_start(out=xt[:, XLEN - 2:XLEN],
                                          in_=zz[:, 0:2])
                        gview = xt[:, LEAD:LEAD + SROWS * PITCH].rearrange(
                            "p (r w) -> p r w", w=PITCH)
                        nc.sync.dma_start(
                            out=gview[:, :, W:PITCH],
                            in_=zz[:, 0:SROWS * 2].rearrange(
                                "p (r w) -> p r w", w=2))
                        nc.sync.dma_start(out=gview[:, :, 0:W],
                                          in_=xsrc[xi][b])
                        x_sb[xi] = xt
                    xt = x_sb[xi]

                    if u == 0:
                        ustore = qstore
                    elif u in (1, 3):
                        ustore = kstorep.tile([128, NTILES, C], BF16,
                                              tag="kT")
                        kT_cur = ustore
                    else:
                        ustore = None

                    if u == 0:
                        g_self = psg.tile([C, C], F32, tag="g")
                    elif u in (1, 3):
                        g_self = psg.tile([C, C], F32, tag="g")
                        g_cross = psg.tile([C, C], F32, tag="g2")

                    for j in range(N_CHUNK):
                        dwps = psdw.tile([C, 2, 512], F32, tag="dwps")
                        for t in range(9):
                            for r2 in range(2):
                                r = 1 + 2 * j + r2
                                off = rowoff(r) + TAPS[t]
                                nc.tensor.matmul(
                                    dwps[:, r2, 0:PITCH],
                                    lhsT=w3_sb[:, u * 9 + t, :],
                                    rhs=xt[:, off:off + PITCH],
                                    start=(t == 0), stop=(t == 8),
                                )
                        if u in (0, 1, 3):
                            dwsb = dwsbp.tile([C, 2, PITCH], BF16)
                            nc.scalar.copy(out=dwsb[:], in_=dwps[:, :, 0:PITCH])
                            tp = pstp.tile([128, 4, C], BF16)
                            for r2 in range(2):
                                for hf in range(2):
                                    nc.tensor.transpose(
                                        tp[:, 2 * r2 + hf, :],
                                        dwsb[:, r2, 128 * hf:128 * hf + 128],
                                        identb[:],
                                    )
                            i0 = 4 * j
                            nc.vector.tensor_copy(
                                out=ustore[:, i0:i0 + 4, :], in_=tp[:])
                            for i in range(i0, i0 + 4):
                                st = (i == 0)
                                sp = (i == 4 * N_CHUNK - 1)
                                if u == 0:
                                    nc.tensor.matmul(
                                        g_self[:], lhsT=qstore[:, i, :],
                                        rhs=qstore[:, i, :],
                                        start=st, stop=sp,
                                        skip_group_check=True)
                                else:
                                    nc.tensor.matmul(
                                        g_cross[:], lhsT=qstore[:, i, :],
                                        rhs=ustore[:, i, :],
                                        start=st, stop=sp,
                                        skip_group_check=True)
                                    nc.tensor.matmul(
                                        g_self[:], lhsT=ustore[:, i, :],
                                        rhs=ustore[:, i, :],
                                        start=st, stop=sp,
                                        skip_group_check=True)
                        else:
                            vslice = v_sum[:, b, :].rearrange(
                                "p (r w) -> p r w", w=PITCH)[:, 2 * j:2 * j + 2, :]
                            if u == 2:
                                nc.scalar.copy(out=vslice,
                                               in_=dwps[:, :, 0:PITCH])
                            else:
                                nc.vector.tensor_add(
                                    out=vslice, in0=dwps[:, :, 0:PITCH],
                                    in1=vslice)
                    # end unit: evacuate gram psums
                    if u == 0:
                        nc.vector.tensor_copy(out=gram_sb[:, b, 0, :],
                                              in_=g_self[:])
                    elif u == 1:
                        nc.vector.tensor_copy(out=gram_sb[:, b, 1, :],
                                              in_=g_cross[:])
                        nc.vector.tensor_copy(out=gram_sb[:, b, 2, :],
                                              in_=g_self[:])
                    elif u == 3:
                        nc.vector.tensor_copy(out=gram_sb[:, b, 3, :],
                                              in_=g_cross[:])
                        nc.vector.tensor_copy(out=gram_sb[:, b, 4, :],
                                              in_=g_self[:])

                if N_UNITS < 5 or SKIP_SM:
                    continue
                # stats: diag extraction via masked reduce
                scr = smallp.tile([C, C], F32, tag="scr")
                for k, slot in enumerate((0, 2, 4)):
                    nc.vector.tensor_mul(out=scr[:],
                                         in0=gram_sb[:, b, slot, :],
                                         in1=ident[:])
                    nc.vector.reduce_sum(out=ar_sb[:, b, 192 + k:193 + k],
                                         in_=scr[:],
                                         axis=mybir.AxisListType.X)
                nc.vector.tensor_copy(out=ar_sb[:, b, 0:96],
                                      in_=gram_sb[:, b, 1, :])
                nc.vector.tensor_copy(out=ar_sb[:, b, 96:192],
                                      in_=gram_sb[:, b, 3, :])

            # ---------------- all-reduce stats ----------------
            import os as _os
            if SKIP_SM:
                pass
            elif _os.environ.get("SKIP_AR"):
                nc.vector.tensor_copy(
                    out=arr_sb[:].rearrange("p a b -> p (a b)"),
                    in_=ar_sb[:].rearrange("p a b -> p (a b)"))
            else:
                nc.sync.dma_start(out=ar_in[:, :],
                                  in_=ar_sb[:].rearrange("p a b -> p (a b)"))
                nc.gpsimd.collective_compute(
                    "AllReduce", mybir.AluOpType.add,
                    replica_groups=[list(range(N_CORES))],
                    ins=[ar_in[:, :]], outs=[ar_out[:, :]],
                )
                nc.sync.dma_start(
                    out=arr_sb[:].rearrange("p a b -> p (a b)"),
                    in_=ar_out[:, :])

            # ---------------- softmax chain ----------------
            for b in range(B if not SKIP_SM else 0):
                rinv = smallp.tile([C, 3], F32, tag="rinv")
                nc.scalar.activation(out=rinv[:], in_=arr_sb[:, b, 192:195],
                                     func=mybir.ActivationFunctionType.Sqrt)
                nc.vector.tensor_scalar_max(out=rinv[:], in0=rinv[:],
                                            scalar1=1e-12)
                nc.vector.reciprocal(out=rinv[:], in_=rinv[:])
                rqt = smallp.tile([C, 1], F32, tag="rqt")
                nc.vector.tensor_mul(out=rqt[:], in0=rinv[:, 0:1],
                                     in1=temp_sb[:])

                ee = smallp.tile([C, 2, C], F32, tag="ee")
                ssum = smallp.tile([C, 2, HEADS], F32, tag="ssum")
                for s in range(2):
                    logits = smallp.tile([C, C], F32, tag="logits")
                    nc.vector.tensor_scalar_mul(
                        out=logits[:], in0=arr_sb[:, b, 96 * s:96 * s + 96],
                        scalar1=rqt[:])
                    # column scale via transpose sandwich:
                    # Lt = L.T ; Lt *= rk (per-partition) ; L = Lt.T
                    lt_ps = psg.tile([C, C], F32, tag="g")
                    nc.tensor.transpose(lt_ps[:], logits[:], ident[:])
                    lts = smallp.tile([C, C], F32, tag="lts")
                    nc.vector.tensor_scalar_mul(out=lts[:], in0=lt_ps[:],
                                                scalar1=rinv[:, 1 + s:2 + s])
                    lt2_ps = psg.tile([C, C], F32, tag="g2")
                    nc.tensor.transpose(lt2_ps[:], lts[:], ident[:])
                    nc.vector.tensor_copy(out=logits[:], in_=lt2_ps[:])
                    nc.scalar.activation(out=ee[:, s, :], in_=logits[:],
                                         func=mybir.ActivationFunctionType.Exp)
                    nc.vector.reduce_sum(
                        out=ssum[:, s, :],
                        in_=ee[:, s, :].rearrange("p (h d) -> p h d", h=HEADS),
                        axis=mybir.AxisListType.X)
                # rpn = 1/(Sp*Sn) per block
                rpn = smallp.tile([C, HEADS], F32, tag="rpn")
                nc.vector.tensor_mul(out=rpn[:], in0=ssum[:, 0, :],
                                     in1=ssum[:, 1, :])
                nc.vector.reciprocal(out=rpn[:], in_=rpn[:])
                # rc[c] = rpn[c, head(c)] via masked reduce
                scrh = smallp.tile([C, HEADS], F32, tag="scrh")
                rc1 = smallp.tile([C, 1], F32, tag="rc1")
                nc.vector.tensor_mul(out=scrh[:], in0=rpn[:], in1=hmask[:])
                nc.vector.reduce_sum(out=rc1[:], in_=scrh[:],
                                     axis=mybir.AxisListType.X)
                pp = smallp.tile([C, C], F32, tag="pp")
                nc.vector.tensor_mul(out=pp[:], in0=ee[:, 0, :],
                                     in1=ee[:, 1, :])
                nc.vector.tensor_scalar_mul(out=pp[:], in0=pp[:],
                                            scalar1=rc1[:])
                e2 = smallp.tile([C, C], F32, tag="e2")
                nc.scalar.activation(out=e2[:], in_=pp[:],
                                     func=mybir.ActivationFunctionType.Exp)
                s2 = smallp.tile([C, HEADS], F32, tag="s2")
                nc.vector.reduce_sum(
                    out=s2[:], in_=e2[:].rearrange("p (h d) -> p h d", h=HEADS),
                    axis=mybir.AxisListType.X)
                nc.vector.reciprocal(out=s2[:], in_=s2[:])
                rc2 = smallp.tile([C, 1], F32, tag="rc2")
                nc.vector.tensor_mul(out=scrh[:], in0=s2[:], in1=hmask[:])
                nc.vector.reduce_sum(out=rc2[:], in_=scrh[:],
                                     axis=mybir.AxisListType.X)
                bd = smallp.tile([C, C], F32, tag="bd")
                nc.vector.tensor_scalar_mul(out=bd[:], in0=e2[:],
                                            scalar1=rc2[:])
                nc.vector.tensor_mul(out=bd[:], in0=bd[:], in1=bmask[:])
                mct_ps = psg.tile([C, C], F32, tag="g2")
                nc.tensor.matmul(mct_ps[:], lhsT=bd[:], rhs=wpt_sb[:],
                                 start=True, stop=True)
                nc.vector.tensor_copy(out=mct_sb[:, b, :], in_=mct_ps[:])

            # ---------------- pass 2: output ----------------
            for b in range(B if not SKIP_SM else 0):
                vview = v_sum[:, b, :].rearrange("p (r w) -> p r w", w=PITCH)
                for j in range(RPC // 2):
                    ops_ = psdw.tile([C, 2, 512], F32, tag="dwps")
                    for r2 in range(2):
                        nc.tensor.matmul(
                            ops_[:, r2, 0:PITCH], lhsT=mct_sb[:, b, :],
                            rhs=vview[:, 2 * j + r2, :], start=True, stop=True)
                    osb = outp.tile([C, 2, PITCH], F32)
                    nc.scalar.copy(out=osb[:], in_=ops_[:, :, 0:PITCH])
                    nc.sync.dma_start(out=y[b, :, 2 * j:2 * j + 2, :],
                                      in_=osb[:, :, 0:W])

    nc.compile()
    return nc


def _prep_inputs(inputs):
    """Build per-core in_maps from full inputs."""
    x_curr = np.asarray(inputs["x_curr"], np.float32)
    x_prev = np.asarray(inputs["x_prev"], np.float32)
    x_next = np.asarray(inputs["x_next"], np.float32)
    w_q = np.asarray(inputs["w_q"], np.float32)
    w_q_dw = np.asarray(inputs["w_q_dw"], np.float32)
    w_kv_prev = np.asarray(inputs["w_kv_prev"], np.float32)
    w_kv_dw_prev = np.asarray(inputs["w_kv_dw_prev"], np.float32)
    w_kv_next = np.asarray(inputs["w_kv_next"], np.float32)
    w_kv_dw_next = np.asarray(inputs["w_kv_dw_next"], np.float32)
    w_proj = np.asarray(inputs["w_proj"], np.float32)
    temperature = np.asarray(inputs["temperature"], np.float32)

    units = [
        (w_q, w_q_dw.reshape(C, 9)),
        (w_kv_prev[0:C], w_kv_dw_prev[0:C].reshape(C, 9)),
        (w_kv_prev[C:2 * C], w_kv_dw_prev[C:2 * C].reshape(C, 9)),
        (w_kv_next[0:C], w_kv_dw_next[0:C].reshape(C, 9)),
        (w_kv_next[C:2 * C], w_kv_dw_next[C:2 * C].reshape(C, 9)),
    ]
    # w3[c, u*9+t, o] = W1_u[o, c] * wdw_u[o, t]
    w3 = np.zeros((C, 45, C), np.float32)
    for u, (w1, wdw) in enumerate(units):
        w3[:, u * 9:(u + 1) * 9, :] = np.einsum("oc,ot->cto", w1, wdw)

    wpt = np.ascontiguousarray(w_proj.T)
    tmpv = np.repeat(temperature.reshape(HEADS), CH).reshape(C, 1)
    tmpv = np.ascontiguousarray(tmpv, np.float32)
    hmk = np.zeros((C, HEADS), np.float32)
    for h in range(HEADS):
        hmk[h * CH:(h + 1) * CH, h] = 1.0
    bmk = np.zeros((C, C), np.float32)
    for h in range(HEADS):
        bmk[h * CH:(h + 1) * CH, h * CH:(h + 1) * CH] = 1.0

    def strip(x, c):
        r0 = c * RPC - 1
        r1 = c * RPC + RPC + 1
        out = np.zeros((B, C, SROWS, W), np.float32)
        lo, hi = max(r0, 0), min(r1, H)
        out[:, :, lo - r0:lo - r0 + hi - lo, :] = x[:, :, lo:hi, :]
        return out

    in_maps = []
    for c in range(N_CORES):
        in_maps.append({
            "xc": strip(x_curr, c),
            "xp": strip(x_prev, c),
            "xn": strip(x_next, c),
            "w3": w3,
            "wpt": wpt,
            "tmp": tmpv,
            "idn": np.eye(C, dtype=np.float32),
            "hmk": hmk,
            "bmk": bmk,
            "zz": np.zeros((C, 128), np.float32),
        })
    return in_maps


def kernel(**inputs):
    if "nc" not in _CACHE:
        _CACHE["nc"] = build_kernel()
    nc = _CACHE["nc"]
    in_maps = _prep_inputs(inputs)
    res = run_bass_kernel_spmd(nc, in_maps, core_ids=list(range(N_CORES)))
    out = np.empty((B, C, H, W), np.float32)
    for c in range(N_CORES):
        out[:, :, c * RPC:(c + 1) * RPC, :] = res.results[c]["y"]
    return out


if __name__ == "__main__":
    rng = np.random.default_rng(0)
    inputs = {
        "x_curr": rng.standard_normal((B, C, H, W), np.float32),
        "x_prev": rng.standard_normal((B, C, H, W), np.float32),
        "x_next": rng.standard_normal((B, C, H, W), np.float32),
        "w_q": rng.standard_normal((C, C), np.float32) * 0.02,
        "w_q_dw": rng.standard_normal((C, 1, 3, 3), np.float32) * 0.02,
        "w_kv_prev": rng.standard_normal((2 * C, C), np.float32) * 0.02,
        "w_kv_dw_prev": rng.standard_normal((2 * C, 1, 3, 3), np.float32) * 0.02,
        "w_kv_next": rng.standard_normal((2 * C, C), np.float32) * 0.02,
        "w_kv_dw_next": rng.standard_normal((2 * C, 1, 3, 3), np.float32) * 0.02,
        "w_proj": rng.standard_normal((C, C), np.float32) * 0.02,
        "temperature": np.ones((HEADS, 1, 1), np.float32),
    }
    out = kernel(**inputs)
    print("out", out.shape, out.dtype, np.abs(out).max())



# revision 15
# speedup vs baseline: 1.5948x; 1.5948x over previous
"""Co-Attention kernel for Trainium2, 8-core SPMD.

Sharding: spatial (H rows) across 8 cores; 32 rows/core with 1-row halo.
Per-core pipeline (all fused, single launch):
  - host pre-builds guarded flat strips (258-pitch rows, zeros in guards):
    fp8e4 2-plane strips (plane1 = flat shift +1) for q/k, bf16 for v
  - conv1x1+dwconv3x3 folded: q/k units run 5 fp8 DoubleRow matmuls per row
    (2 taps per pass: 3 horizontal pairs via the +1 plane, 1 vertical pair
    via a stride-258 view, 1 zero-padded), v units run 9 bf16 matmuls
  - q/k weights are rescaled per unit into fp8 range; the L2 normalization
    of q/k makes the channel attention invariant to that scale
  - q/k: PE transpose -> fp8 [n,c] tiles -> Gram matrices via fp8 DoubleRow
    (2 spatial tiles per pass), accumulated over the core's spatial shard
  - v: v_prev+v_next accumulated into an SBUF-resident bf16 strip
  - AllReduce of the tiny Gram/norm stats across the 8 cores
  - on-chip double softmax (block-diagonal channel attention)
  - output = (w_proj @ blockdiag(attn_co)) @ v_sum, one matmul per chunk,
    DMA'd to DRAM directly from PSUM
"""

import sys

sys.path.insert(0, "/opt/trn_rl_repo")

import ml_dtypes
import numpy as np

import concourse.bacc as bacc
import concourse.bass as bass
import concourse.tile as tile
from concourse import mybir
from concourse.bass_utils import run_bass_kernel_spmd

# problem constants
B, C, H, W = 2, 96, 256, 256
HEADS = 4
CH = C // HEADS
N_CORES = 8
RPC = H // N_CORES          # rows per core (32)
SROWS = RPC + 2             # strip rows incl halo (34)
PITCH = W + 2               # guarded row pitch (258)
LEAD = 2                    # leading guard pad
XLEN = LEAD + SROWS * PITCH + 2  # strip flat length (8776)
NTILES = RPC * 2            # 128-wide transpose tiles per unit per b (64)
VLEN = RPC * PITCH          # v_sum flat length per b (8256)

F32 = mybir.dt.float32
F32R = mybir.dt.float32r
BF16 = mybir.dt.bfloat16
F8 = mybir.dt.float8e4
DR = mybir.MatmulPerfMode.DoubleRow

NP_F8 = ml_dtypes.float8_e4m3
NP_BF16 = ml_dtypes.bfloat16

# tap offsets (cross-correlation, matching jax.lax.conv_general_dilated)
TAPS = [(ky - 1) * PITCH + (kx - 1) for ky in range(3) for kx in range(3)]
# DoubleRow tap pairs for q/k: ('A' = plane pair at +1, 'B' = vertical
# stride-258 pair on plane0, second tap None = zero-padded plane pair)
PAIRS = [(0, 1, "A"), (3, 4, "A"), (6, 7, "A"), (2, 5, "B"), (8, None, "A")]

_CACHE = {}


def rowoff(r):
    return LEAD + r * PITCH


def build_kernel():
    import os as _os
    N_UNITS = int(_os.environ.get("N_UNITS", "5"))
    N_B = int(_os.environ.get("N_B", str(B)))
    N_CHUNK = int(_os.environ.get("N_CHUNK", str(RPC // 2)))
    SKIP_SM = bool(_os.environ.get("SKIP_SM"))
    nc = bacc.Bacc("TRN2", target_bir_lowering=False, debug=False,
                   num_devices=N_CORES)

    xc8 = nc.declare_dram_parameter("xc8", [B, C, 2, XLEN], F8, isOutput=False)
    xp8 = nc.declare_dram_parameter("xp8", [B, C, 2, XLEN], F8, isOutput=False)
    xn8 = nc.declare_dram_parameter("xn8", [B, C, 2, XLEN], F8, isOutput=False)
    xpv = nc.declare_dram_parameter("xpv", [B, C, XLEN], BF16, isOutput=False)
    xnv = nc.declare_dram_parameter("xnv", [B, C, XLEN], BF16, isOutput=False)
    w8 = nc.declare_dram_parameter("w8", [C, 15, 2, C], F8, isOutput=False)
    w3v = nc.declare_dram_parameter("w3v", [C, 18, C], BF16, isOutput=False)
    wpt = nc.declare_dram_parameter("wpt", [C, C], F32, isOutput=False)
    tmp = nc.declare_dram_parameter("tmp", [C, 1], F32, isOutput=False)
    idn = nc.declare_dram_parameter("idn", [C, C], F32, isOutput=False)
    hmk = nc.declare_dram_parameter("hmk", [C, HEADS], F32, isOutput=False)
    bmk = nc.declare_dram_parameter("bmk", [C, C], F32, isOutput=False)
    y = nc.declare_dram_parameter("y", [B, C, RPC, W], F32, isOutput=True)

    ar_in = nc.dram_tensor("ar_in", [C, 2 * 195], F32)
    ar_out = nc.dram_tensor("ar_out", [C, 2 * 195], F32, addr_space="Shared")

    x8src = {0: xc8, 1: xp8, 2: xn8}
    xvsrc = {1: xpv, 2: xnv}

    with tile.TileContext(nc) as tc:
        with (
            tc.tile_pool(name="singles", bufs=1) as singles,
            tc.tile_pool(name="x8pool", bufs=2) as x8pool,
            tc.tile_pool(name="xvpool", bufs=2) as xvpool,
            tc.tile_pool(name="dwsb", bufs=3) as dwsbp,
            tc.tile_pool(name="store", bufs=1) as storep,
            tc.tile_pool(name="kstore", bufs=2) as kstorep,
            tc.tile_pool(name="small", bufs=4) as smallp,
            tc.tile_pool(name="outp", bufs=3) as outp,
            tc.tile_pool(name="psdw", bufs=2, space="PSUM") as psdw,
            tc.tile_pool(name="pstp", bufs=2, space="PSUM") as pstp,
            tc.tile_pool(name="psg", bufs=1, space="PSUM") as psg,
        ):
            # ---- constants ----
            w8_sb = singles.tile([C, 15, 2, C], F8)
            nc.sync.dma_start(out=w8_sb[:], in_=w8[:, :, :, :])
            w3v_sb = singles.tile([C, 18, C], BF16)
            nc.sync.dma_start(out=w3v_sb[:], in_=w3v[:, :, :])
            wpt_sb = singles.tile([C, C], F32)
            nc.sync.dma_start(out=wpt_sb[:], in_=wpt[:, :])
            temp_sb = singles.tile([C, 1], F32)
            nc.sync.dma_start(out=temp_sb[:], in_=tmp[:, :])
            ident = singles.tile([C, C], F32)
            nc.sync.dma_start(out=ident[:], in_=idn[:, :])
            identb = singles.tile([C, C], BF16)
            nc.vector.tensor_copy(out=identb[:], in_=ident[:])
            hmask = singles.tile([C, HEADS], F32)
            nc.sync.dma_start(out=hmask[:], in_=hmk[:, :])
            bmask = singles.tile([C, C], F32)
            nc.sync.dma_start(out=bmask[:], in_=bmk[:, :])

            # persistent accumulators
            v_sum = singles.tile([C, B, VLEN], BF16)
            ar_sb = singles.tile([C, B, 195], F32)
            gram_sb = singles.tile([C, B, 5, C], F32)
            arr_sb = singles.tile([C, B, 195], F32)
            mct_sb = singles.tile([C, B, C], BF16)

            qstore = storep.tile([128, NTILES, C], F8)

            # ---------------- pass 1: conv + dw + grams + v_sum ----------
            for b in range(N_B):
                x8_sb = {}
                xv_sb = {}
                for u in range(N_UNITS):
                    xi = [0, 1, 1, 2, 2][u]
                    qk = u in (0, 1, 3)
                    if qk:
                        if xi not in x8_sb:
                            t8 = x8pool.tile([C, 2, XLEN], F8, tag="x8")
                            nc.sync.dma_start(out=t8[:], in_=x8src[xi][b])
                            x8_sb[xi] = t8
                        xt8 = x8_sb[xi]
                        slot = {0: 0, 1: 1, 3: 2}[u]
                    else:
                        if xi not in xv_sb:
                            tv = xvpool.tile([C, XLEN], BF16, tag="xv")
                            nc.sync.dma_start(out=tv[:], in_=xvsrc[xi][b])
                            xv_sb[xi] = tv
                        xtv = xv_sb[xi]
                        wbase = 0 if u == 2 else 9

                    if u == 0:
                        ustore = qstore
                    elif u in (1, 3):
                        ustore = kstorep.tile([128, NTILES, C], F8, tag="kT")
                    else:
                        ustore = None

                    if u == 0:
                        g_self = psg.tile([C, C], F32, tag="g")
                    elif u in (1, 3):
                        g_self = psg.tile([C, C], F32, tag="g")
                        g_cross = psg.tile([C, C], F32, tag="g2")

                    for j in range(N_CHUNK):
                        dwps = psdw.tile([C, 2, 512], F32, tag="dwps")
                        for r2 in range(2):
                            r = 1 + 2 * j + r2
                            ro = rowoff(r)
                            if qk:
                                for p, (t1, t2, kind) in enumerate(PAIRS):
                                    a = ro + TAPS[t1]
                                    if kind == "A":
                                        rhs = xt8[:, :, a:a + PITCH]
                                    else:
                                        rhs = xt8[:, 0:1, a:a + 2 * PITCH]\
                                            .rearrange(
                                                "p one (two n) -> p (one two) n",
                                                two=2)
                                    nc.tensor.matmul(
                                        dwps[:, r2, 0:PITCH],
                                        lhsT=w8_sb[:, slot * 5 + p, :, :],
                                        rhs=rhs,
                                        start=(p == 0), stop=(p == 4),
                                        perf_mode=DR,
                                    )
                            else:
                                for t in range(9):
                                    a = ro + TAPS[t]
                                    nc.tensor.matmul(
                                        dwps[:, r2, 0:PITCH],
                                        lhsT=w3v_sb[:, wbase + t, :],
                                        rhs=xtv[:, a:a + PITCH],
                                        start=(t == 0), stop=(t == 8),
                                    )
                        if qk:
                            dwsb = dwsbp.tile([C, 2, PITCH], BF16)
                            nc.scalar.copy(out=dwsb[:], in_=dwps[:, :, 0:PITCH])
                            tp = pstp.tile([128, 4, C], BF16)
                            for r2 in range(2):
                                for hf in range(2):
                                    nc.tensor.transpose(
                                        tp[:, 2 * r2 + hf, :],
                                        dwsb[:, r2, 128 * hf:128 * hf + 128],
                                        identb[:],
                                    )
                            i0 = 4 * j
                            nc.vector.tensor_copy(
                                out=ustore[:, i0:i0 + 4, :], in_=tp[:])
                            for i in (i0, i0 + 2):
                                st = (i == 0)
                                sp = (i == 4 * N_CHUNK - 2)
                                if u == 0:
                                    nc.tensor.matmul(
                                        g_self[:],
                                        lhsT=qstore[:, i:i + 2, :],
                                        rhs=qstore[:, i:i + 2, :],
                                        start=st, stop=sp, perf_mode=DR,
                                        skip_group_check=True)
                                else:
                                    nc.tensor.matmul(
                                        g_cross[:],
                                        lhsT=qstore[:, i:i + 2, :],
                                        rhs=ustore[:, i:i + 2, :],
                                        start=st, stop=sp, perf_mode=DR,
                                        skip_group_check=True)
                                    nc.tensor.matmul(
                                        g_self[:],
                                        lhsT=ustore[:, i:i + 2, :],
                                        rhs=ustore[:, i:i + 2, :],
                                        start=st, stop=sp, perf_mode=DR,
                                        skip_group_check=True)
                        else:
                            vslice = v_sum[:, b, :].rearrange(
                                "p (r w) -> p r w", w=PITCH)[:, 2 * j:2 * j + 2, :]
                            if u == 2:
                                nc.vector.tensor_copy(
                                    out=vslice, in_=dwps[:, :, 0:PITCH])
                            else:
                                nc.vector.tensor_add(
                                    out=vslice, in0=dwps[:, :, 0:PITCH],
                                    in1=vslice)
                    # end unit: evacuate gram psums
                    if u == 0:
                        nc.vector.tensor_copy(out=gram_sb[:, b, 0, :],
                                              in_=g_self[:])
                    elif u == 1:
                        nc.vector.tensor_copy(out=gram_sb[:, b, 1, :],
                                              in_=g_cross[:])
                        nc.vector.tensor_copy(out=gram_sb[:, b, 2, :],
                                              in_=g_self[:])
                    elif u == 3:
                        nc.vector.tensor_copy(out=gram_sb[:, b, 3, :],
                                              in_=g_cross[:])
                        nc.vector.tensor_copy(out=gram_sb[:, b, 4, :],
                                              in_=g_self[:])

                if N_UNITS < 5 or SKIP_SM:
                    continue
                # stats: diag extraction via masked reduce
                scr = smallp.tile([C, C], F32, tag="scr")
                for k, slot_ in enumerate((0, 2, 4)):
                    nc.vector.tensor_mul(out=scr[:],
                                         in0=gram_sb[:, b, slot_, :],
                                         in1=ident[:])
                    nc.vector.reduce_sum(out=ar_sb[:, b, 192 + k:193 + k],
                                         in_=scr[:],
                                         axis=mybir.AxisListType.X)
                nc.vector.tensor_copy(out=ar_sb[:, b, 0:96],
                                      in_=gram_sb[:, b, 1, :])
                nc.vector.tensor_copy(out=ar_sb[:, b, 96:192],
                                      in_=gram_sb[:, b, 3, :])

            # ---------------- all-reduce stats ----------------
            import os as _os
            if SKIP_SM:
                pass
            elif _os.environ.get("SKIP_AR"):
                nc.vector.tensor_copy(
                    out=arr_sb[:].rearrange("p a b -> p (a b)"),
                    in_=ar_sb[:].rearrange("p a b -> p (a b)"))
            else:
                nc.sync.dma_start(out=ar_in[:, :],
                                  in_=ar_sb[:].rearrange("p a b -> p (a b)"))
                nc.gpsimd.collective_compute(
                    "AllReduce", mybir.AluOpType.add,
                    replica_groups=[list(range(N_CORES))],
                    ins=[ar_in[:, :]], outs=[ar_out[:, :]],
                )
                nc.sync.dma_start(
                    out=arr_sb[:].rearrange("p a b -> p (a b)"),
                    in_=ar_out[:, :])

            # ---------------- softmax chain ----------------
            for b in range(B if not SKIP_SM else 0):
                rinv = smallp.tile([C, 3], F32, tag="rinv")
                nc.scalar.activation(out=rinv[:], in_=arr_sb[:, b, 192:195],
                                     func=mybir.ActivationFunctionType.Sqrt)
                nc.vector.tensor_scalar_max(out=rinv[:], in0=rinv[:],
                                            scalar1=1e-12)
                nc.vector.reciprocal(out=rinv[:], in_=rinv[:])
                rqt = smallp.tile([C, 1], F32, tag="rqt")
                nc.vector.tensor_mul(out=rqt[:], in0=rinv[:, 0:1],
                                     in1=temp_sb[:])

                ee = smallp.tile([C, 2, C], F32, tag="ee")
                ssum = smallp.tile([C, 2, HEADS], F32, tag="ssum")
                for s in range(2):
                    logits = smallp.tile([C, C], F32, tag="logits")
                    nc.vector.tensor_scalar_mul(
                        out=logits[:], in0=arr_sb[:, b, 96 * s:96 * s + 96],
                        scalar1=rqt[:])
                    # column scale via transpose sandwich:
                    # Lt = L.T ; Lt *= rk (per-partition) ; L = Lt.T
                    lt_ps = psg.tile([C, C], F32, tag="g")
                    nc.tensor.transpose(lt_ps[:], logits[:], ident[:])
                    lts = smallp.tile([C, C], F32, tag="lts")
                    nc.vector.tensor_scalar_mul(out=lts[:], in0=lt_ps[:],
                                                scalar1=rinv[:, 1 + s:2 + s])
                    lt2_ps = psg.tile([C, C], F32, tag="g2")
                    nc.tensor.transpose(lt2_ps[:], lts[:], ident[:])
                    nc.vector.tensor_copy(out=logits[:], in_=lt2_ps[:])
                    nc.scalar.activation(out=ee[:, s, :], in_=logits[:],
                                         func=mybir.ActivationFunctionType.Exp)
                    nc.vector.reduce_sum(
                        out=ssum[:, s, :],
                        in_=ee[:, s, :].rearrange("p (h d) -> p h d", h=HEADS),
                        axis=mybir.AxisListType.X)
                # rpn = 1/(Sp*Sn) per block
                rpn = smallp.tile([C, HEADS], F32, tag="rpn")
                nc.vector.tensor_mul(out=rpn[:], in0=ssum[:, 0, :],
                                     in1=ssum[:, 1, :])
                nc.vector.reciprocal(out=rpn[:], in_=rpn[:])
                # rc[c] = rpn[c, head(c)] via masked reduce
                scrh = smallp.tile([C, HEADS], F32, tag="scrh")
                rc1 = smallp.tile([C, 1], F32, tag="rc1")
                nc.vector.tensor_mul(out=scrh[:], in0=rpn[:], in1=hmask[:])
                nc.vector.reduce_sum(out=rc1[:], in_=scrh[:],
                                     axis=mybir.AxisListType.X)
                pp = smallp.tile([C, C], F32, tag="pp")
                nc.vector.tensor_mul(out=pp[:], in0=ee[:, 0, :],
                                     in1=ee[:, 1, :])
                nc.vector.tensor_scalar_mul(out=pp[:], in0=pp[:],
                                            scalar1=rc1[:])
                e2 = smallp.tile([C, C], F32, tag="e2")
                nc.scalar.activation(out=e2[:], in_=pp[:],
                                     func=mybir.ActivationFunctionType.Exp)
                s2 = smallp.tile([C, HEADS], F32, tag="s2")
                nc.vector.reduce_sum(
                    out=s2[:], in_=e2[:].rearrange("p (h d) -> p h d", h=HEADS),
                    axis=mybir.AxisListType.X)
                nc.vector.reciprocal(out=s2[:], in_=s2[:])
                rc2 = smallp.tile([C, 1], F32, tag="rc2")
                nc.vector.tensor_mul(out=scrh[:], in0=s2[:], in1=hmask[:])
                nc.vector.reduce_sum(out=rc2[:], in_=scrh[:],
                                     axis=mybir.AxisListType.X)
                bd = smallp.tile([C, C], F32, tag="bd")
                nc.vector.tensor_scalar_mul(out=bd[:], in0=e2[:],
                                            scalar1=rc2[:])
                nc.vector.tensor_mul(out=bd[:], in0=bd[:], in1=bmask[:])
                mct_ps = psg.tile([C, C], F32, tag="g2")
                nc.tensor.matmul(mct_ps[:], lhsT=bd[:], rhs=wpt_sb[:],
                                 start=True, stop=True)
                nc.vector.tensor_copy(out=mct_sb[:, b, :], in_=mct_ps[:])

            # ---------------- pass 2: output ----------------
            for b in range(B if not SKIP_SM else 0):
                vview = v_sum[:, b, :].rearrange("p (r w) -> p r w", w=PITCH)
                for j in range(RPC // 2):
                    ops_ = psdw.tile([C, 2, 512], F32, tag="dwps")
                    for r2 in range(2):
                        nc.tensor.matmul(
                            ops_[:, r2, 0:PITCH], lhsT=mct_sb[:, b, :],
                            rhs=vview[:, 2 * j + r2, :], start=True, stop=True)
                    osb = outp.tile([C, 2, PITCH], F32)
                    nc.scalar.copy(out=osb[:], in_=ops_[:, :, 0:PITCH])
                    nc.sync.dma_start(out=y[b, :, 2 * j:2 * j + 2, :],
                                      in_=osb[:, :, 0:W])

    nc.compile()
    return nc


def _flat_strip(x, c):
    """Guarded flat strip [B, C, XLEN] (fp32) for core c."""
    r0 = c * RPC - 1
    lo, hi = max(r0, 0), min(r0 + SROWS, H)
    body = np.zeros((B, C, SROWS, PITCH), np.float32)
    body[:, :, lo - r0:lo - r0 + hi - lo, 0:W] = x[:, :, lo:hi, :]
    out = np.zeros((B, C, XLEN), np.float32)
    out[:, :, LEAD:LEAD + SROWS * PITCH] = body.reshape(B, C, -1)
    return out


def _two_plane_f8(flat):
    """[B, C, XLEN] fp32 -> [B, C, 2, XLEN] fp8e4 (plane1 = flat shift +1)."""
    p1 = np.zeros_like(flat)
    p1[:, :, :XLEN - 1] = flat[:, :, 1:]
    return np.stack([flat, p1], axis=2).astype(NP_F8)


def _prep_inputs(inputs):
    """Build per-core in_maps from full inputs."""
    x_curr = np.asarray(inputs["x_curr"], np.float32)
    x_prev = np.asarray(inputs["x_prev"], np.float32)
    x_next = np.asarray(inputs["x_next"], np.float32)
    w_q = np.asarray(inputs["w_q"], np.float32)
    w_q_dw = np.asarray(inputs["w_q_dw"], np.float32)
    w_kv_prev = np.asarray(inputs["w_kv_prev"], np.float32)
    w_kv_dw_prev = np.asarray(inputs["w_kv_dw_prev"], np.float32)
    w_kv_next = np.asarray(inputs["w_kv_next"], np.float32)
    w_kv_dw_next = np.asarray(inputs["w_kv_dw_next"], np.float32)
    w_proj = np.asarray(inputs["w_proj"], np.float32)
    temperature = np.asarray(inputs["temperature"], np.float32)

    units = [
        (w_q, w_q_dw.reshape(C, 9)),
        (w_kv_prev[0:C], w_kv_dw_prev[0:C].reshape(C, 9)),
        (w_kv_prev[C:2 * C], w_kv_dw_prev[C:2 * C].reshape(C, 9)),
        (w_kv_next[0:C], w_kv_dw_next[0:C].reshape(C, 9)),
        (w_kv_next[C:2 * C], w_kv_dw_next[C:2 * C].reshape(C, 9)),
    ]
    # w3[u][c, t, o] = W1_u[o, c] * wdw_u[o, t]
    w3 = [np.einsum("oc,ot->cto", w1, wdw).astype(np.float32)
          for (w1, wdw) in units]

    # q/k units -> fp8 DoubleRow pair weights, rescaled into fp8 range
    # (scale cancels in the channel-attention L2 normalization)
    w8 = np.zeros((C, 15, 2, C), np.float32)
    for si, u in enumerate((0, 1, 3)):
        wu = w3[u]
        denom = np.sqrt((wu.astype(np.float64) ** 2).sum(axis=(0, 1)).mean())
        s = 16.0 / max(denom, 1e-30)
        for p, (t1, t2, _) in enumerate(PAIRS):
            w8[:, si * 5 + p, 0, :] = wu[:, t1, :] * s
            if t2 is not None:
                w8[:, si * 5 + p, 1, :] = wu[:, t2, :] * s
    w8 = w8.astype(NP_F8)

    # v units -> bf16 plain taps
    w3v = np.concatenate([w3[2], w3[4]], axis=1).astype(NP_BF16)

    wpt = np.ascontiguousarray(w_proj.T)
    tmpv = np.repeat(temperature.reshape(HEADS), CH).reshape(C, 1)
    tmpv = np.ascontiguousarray(tmpv, np.float32)
    hmk = np.zeros((C, HEADS), np.float32)
    for h in range(HEADS):
        hmk[h * CH:(h + 1) * CH, h] = 1.0
    bmk = np.zeros((C, C), np.float32)
    for h in range(HEADS):
        bmk[h * CH:(h + 1) * CH, h * CH:(h + 1) * CH] = 1.0

    in_maps = []
    for c in range(N_CORES):
        fc = _flat_strip(x_curr, c)
        fp = _flat_strip(x_prev, c)
        fn = _flat_strip(x_next, c)
        in_maps.append({
            "xc8": _two_plane_f8(fc),
            "xp8": _two_plane_f8(fp),
            "xn8": _two_plane_f8(fn),
            "xpv": fp.astype(NP_BF16),
            "xnv": fn.astype(NP_BF16),
            "w8": w8,
            "w3v": w3v,
            "wpt": wpt,
            "tmp": tmpv,
            "idn": np.eye(C, dtype=np.float32),
            "hmk": hmk,
            "bmk": bmk,
        })
    return in_maps


def kernel(**inputs):
    if "nc" not in _CACHE:
        _CACHE["nc"] = build_kernel()
    nc = _CACHE["nc"]
    in_maps = _prep_inputs(inputs)
    res = run_bass_kernel_spmd(nc, in_maps, core_ids=list(range(N_CORES)))
    out = np.empty((B, C, H, W), np.float32)
    for c in range(N_CORES):
        out[:, :, c * RPC:(c + 1) * RPC, :] = res.results[c]["y"]
    return out


if __name__ == "__main__":
    rng = np.random.default_rng(0)
    inputs = {
        "x_curr": rng.standard_normal((B, C, H, W), np.float32),
        "x_prev": rng.standard_normal((B, C, H, W), np.float32),
        "x_next": rng.standard_normal((B, C, H, W), np.float32),
        "w_q": rng.standard_normal((C, C), np.float32) * 0.02,
        "w_q_dw": rng.standard_normal((C, 1, 3, 3), np.float32) * 0.02,
        "w_kv_prev": rng.standard_normal((2 * C, C), np.float32) * 0.02,
        "w_kv_dw_prev": rng.standard_normal((2 * C, 1, 3, 3), np.float32) * 0.02,
        "w_kv_next": rng.standard_normal((2 * C, C), np.float32) * 0.02,
        "w_kv_dw_next": rng.standard_normal((2 * C, 1, 3, 3), np.float32) * 0.02,
        "w_proj": rng.standard_normal((C, C), np.float32) * 0.02,
        "temperature": np.ones((HEADS, 1, 1), np.float32),
    }
    out = kernel(**inputs)
    print("out", out.shape, out.dtype, np.abs(out).max())


# revision 25
# speedup vs baseline: 1.7001x; 1.0660x over previous
"""Co-Attention kernel for Trainium2, 8-core SPMD.

Sharding: spatial (H rows) across 8 cores; 32 rows/core with 1-row halo.
Per-core pipeline (all fused, single launch):
  - host pre-builds guarded flat strips (258-pitch rows, zeros in guards):
    fp8e4 2-plane strips (plane1 = flat shift +1) for q/k, bf16 for v
  - conv1x1+dwconv3x3 folded: q/k units run 5 fp8 DoubleRow matmuls per row
    (2 taps per pass: 3 horizontal pairs via the +1 plane, 1 vertical pair
    via a stride-258 view, 1 zero-padded), v units run 9 bf16 matmuls
  - q/k weights are rescaled per unit into fp8 range; the L2 normalization
    of q/k makes the channel attention invariant to that scale
  - q/k: PE transpose -> fp8 [n,c] tiles -> Gram matrices via fp8 DoubleRow
    (2 spatial tiles per pass), accumulated over the core's spatial shard
  - v: v_prev+v_next accumulated into an SBUF-resident bf16 strip
  - AllReduce of the tiny Gram/norm stats across the 8 cores
  - on-chip double softmax (block-diagonal channel attention)
  - output = (w_proj @ blockdiag(attn_co)) @ v_sum, one matmul per chunk,
    DMA'd to DRAM directly from PSUM
"""

import sys

sys.path.insert(0, "/opt/trn_rl_repo")

import ml_dtypes
import numpy as np

import concourse.bacc as bacc
import concourse.bass as bass
import concourse.tile as tile
from concourse import mybir
from concourse.bass_utils import run_bass_kernel_spmd

# problem constants
B, C, H, W = 2, 96, 256, 256
HEADS = 4
CH = C // HEADS
N_CORES = 8
RPC = H // N_CORES          # rows per core (32)
SROWS = RPC + 2             # strip rows incl halo (34)
PITCH = W + 2               # guarded row pitch (258)
LEAD = 2                    # leading guard pad
XLEN = LEAD + SROWS * PITCH + 2  # strip flat length (8776)
NTILES = RPC * 2            # 128-wide transpose tiles per unit per b (64)
VLEN = RPC * PITCH          # v_sum flat length per b (8256)

F32 = mybir.dt.float32
F32R = mybir.dt.float32r
BF16 = mybir.dt.bfloat16
F8 = mybir.dt.float8e4
DR = mybir.MatmulPerfMode.DoubleRow

NP_F8 = ml_dtypes.float8_e4m3
NP_BF16 = ml_dtypes.bfloat16

# tap offsets (cross-correlation, matching jax.lax.conv_general_dilated)
TAPS = [(ky - 1) * PITCH + (kx - 1) for ky in range(3) for kx in range(3)]
# DoubleRow tap pairs for q/k: ('A' = plane pair at +1, 'B' = vertical
# stride-258 pair on plane0, second tap None = zero-padded plane pair)
PAIRS = [(0, 1, "A"), (3, 4, "A"), (6, 7, "A"), (2, 5, "B"), (8, None, "A")]
# strip DMA split points (flat offsets; 3 pieces so conv can start after
# the first rows land)
_P1 = 2 + 12 * PITCH
_P2 = 2 + 23 * PITCH
DMA_PIECES = [(0, _P1), (_P1, _P2), (_P2, XLEN)]

_CACHE = {}


def rowoff(r):
    return LEAD + r * PITCH


def build_kernel():
    import os as _os
    N_UNITS = int(_os.environ.get("N_UNITS", "5"))
    N_B = int(_os.environ.get("N_B", str(B)))
    N_CHUNK = int(_os.environ.get("N_CHUNK", str(RPC // 2)))
    SKIP_SM = bool(_os.environ.get("SKIP_SM"))
    nc = bacc.Bacc("TRN2", target_bir_lowering=False, debug=False,
                   num_devices=N_CORES)

    xc8 = nc.declare_dram_parameter("xc8", [B, C, 2, XLEN], F8, isOutput=False)
    xp8 = nc.declare_dram_parameter("xp8", [B, C, 2, XLEN], F8, isOutput=False)
    xn8 = nc.declare_dram_parameter("xn8", [B, C, 2, XLEN], F8, isOutput=False)
    xpv = nc.declare_dram_parameter("xpv", [B, C, XLEN], BF16, isOutput=False)
    xnv = nc.declare_dram_parameter("xnv", [B, C, XLEN], BF16, isOutput=False)
    w8 = nc.declare_dram_parameter("w8", [C, 15, 2, C], F8, isOutput=False)
    w3v = nc.declare_dram_parameter("w3v", [C, 18, C], BF16, isOutput=False)
    wpt = nc.declare_dram_parameter("wpt", [C, C], F32, isOutput=False)
    tmp = nc.declare_dram_parameter("tmp", [C, 1], F32, isOutput=False)
    idn = nc.declare_dram_parameter("idn", [C, C], F32, isOutput=False)
    hmk = nc.declare_dram_parameter("hmk", [C, HEADS], F32, isOutput=False)
    bmk = nc.declare_dram_parameter("bmk", [C, C], F32, isOutput=False)
    y = nc.declare_dram_parameter("y", [B, C, RPC, W], BF16, isOutput=True)

    ar_in = [nc.dram_tensor(f"ar_in{i}", [C, 195], F32) for i in range(B)]
    ar_out = [nc.dram_tensor(f"ar_out{i}", [C, 195], F32, addr_space="Shared")
              for i in range(B)]

    x8src = {0: xc8, 1: xp8, 2: xn8}
    xvsrc = {1: xpv, 2: xnv}

    with tile.TileContext(nc) as tc:
        with (
            tc.tile_pool(name="singles", bufs=1) as singles,
            tc.tile_pool(name="x8pool", bufs=2) as x8pool,
            tc.tile_pool(name="xvpool", bufs=2) as xvpool,
            tc.tile_pool(name="dwsb", bufs=3) as dwsbp,
            tc.tile_pool(name="store", bufs=1) as storep,
            tc.tile_pool(name="kstore", bufs=2) as kstorep,
            tc.tile_pool(name="small", bufs=4) as smallp,
            tc.tile_pool(name="outp", bufs=3) as outp,
            tc.tile_pool(name="psdw", bufs=2, space="PSUM") as psdw,
            tc.tile_pool(name="pstp", bufs=2, space="PSUM") as pstp,
            tc.tile_pool(name="psg", bufs=1, space="PSUM") as psg,
        ):
            # ---- constants ----
            w8_sb = singles.tile([C, 15, 2, C], F8)
            nc.sync.dma_start(out=w8_sb[:], in_=w8[:, :, :, :])
            w3v_sb = singles.tile([C, 18, C], BF16)
            nc.sync.dma_start(out=w3v_sb[:], in_=w3v[:, :, :])
            wpt_sb = singles.tile([C, C], F32)
            nc.sync.dma_start(out=wpt_sb[:], in_=wpt[:, :])
            temp_sb = singles.tile([C, 1], F32)
            nc.sync.dma_start(out=temp_sb[:], in_=tmp[:, :])
            ident = singles.tile([C, C], F32)
            nc.sync.dma_start(out=ident[:], in_=idn[:, :])
            identb = singles.tile([C, C], BF16)
            nc.vector.tensor_copy(out=identb[:], in_=ident[:])
            hmask = singles.tile([C, HEADS], F32)
            nc.sync.dma_start(out=hmask[:], in_=hmk[:, :])
            bmask = singles.tile([C, C], F32)
            nc.sync.dma_start(out=bmask[:], in_=bmk[:, :])

            # persistent accumulators
            v_sum = singles.tile([C, B, VLEN], BF16)
            ar_sb = singles.tile([C, B, 195], F32)
            gram_sb = singles.tile([C, B, 5, C], F32)
            arr_sb = singles.tile([C, B, 195], F32)
            mct_sb = singles.tile([C, B, C], BF16)

            qstore = storep.tile([128, NTILES, C], F8)

            # ---------------- helper blocks ----------------
            def stats_ar(b):
                """Diag stats + per-b AllReduce (issued right after b's
                pass 1 so the collective overlaps the next b's compute)."""
                scr = smallp.tile([C, C], F32, tag="scr")
                for k, slot_ in enumerate((0, 2, 4)):
                    nc.vector.tensor_mul(out=scr[:],
                                         in0=gram_sb[:, b, slot_, :],
                                         in1=ident[:])
                    nc.vector.reduce_sum(out=ar_sb[:, b, 192 + k:193 + k],
                                         in_=scr[:],
                                         axis=mybir.AxisListType.X)
                nc.vector.tensor_copy(out=ar_sb[:, b, 0:96],
                                      in_=gram_sb[:, b, 1, :])
                nc.vector.tensor_copy(out=ar_sb[:, b, 96:192],
                                      in_=gram_sb[:, b, 3, :])
                import os as _os
                if _os.environ.get("SKIP_AR"):
                    nc.vector.tensor_copy(out=arr_sb[:, b, :],
                                          in_=ar_sb[:, b, :])
                else:
                    nc.sync.dma_start(out=ar_in[b][:, :], in_=ar_sb[:, b, :])
                    nc.gpsimd.collective_compute(
                        "AllReduce", mybir.AluOpType.add,
                        replica_groups=[list(range(N_CORES))],
                        ins=[ar_in[b][:, :]], outs=[ar_out[b][:, :]],
                    )
                    nc.sync.dma_start(out=arr_sb[:, b, :], in_=ar_out[b][:, :])

            def softmax_pass2(b):
                rinv = smallp.tile([C, 3], F32, tag="rinv")
                nc.scalar.activation(out=rinv[:], in_=arr_sb[:, b, 192:195],
                                     func=mybir.ActivationFunctionType.Sqrt)
                nc.vector.tensor_scalar_max(out=rinv[:], in0=rinv[:],
                                            scalar1=1e-12)
                nc.vector.reciprocal(out=rinv[:], in_=rinv[:])
                rqt = smallp.tile([C, 1], F32, tag="rqt")
                nc.vector.tensor_mul(out=rqt[:], in0=rinv[:, 0:1],
                                     in1=temp_sb[:])

                ee = smallp.tile([C, 2, C], F32, tag="ee")
                ssum = smallp.tile([C, 2, HEADS], F32, tag="ssum")
                logits = smallp.tile([C, 2, C], F32, tag="logits")
                nc.vector.tensor_scalar_mul(
                    out=logits[:],
                    in0=arr_sb[:, b, 0:192].rearrange("p (s c) -> p s c", s=2),
                    scalar1=rqt[:])
                # column scale via transpose sandwich, both branches at once:
                # Lt = L.T ; Lt *= rk (per-partition) ; L = Lt.T
                lt_ps = psg.tile([C, 2, C], F32, tag="g")
                for s in range(2):
                    nc.tensor.transpose(lt_ps[:, s, :], logits[:, s, :],
                                        ident[:])
                lts = smallp.tile([C, 2, C], F32, tag="lts")
                for s in range(2):
                    nc.vector.tensor_scalar_mul(out=lts[:, s, :],
                                                in0=lt_ps[:, s, :],
                                                scalar1=rinv[:, 1 + s:2 + s])
                lt2_ps = psg.tile([C, 2, C], F32, tag="g2")
                for s in range(2):
                    nc.tensor.transpose(lt2_ps[:, s, :], lts[:, s, :],
                                        ident[:])
                nc.scalar.activation(out=ee[:], in_=lt2_ps[:],
                                     func=mybir.ActivationFunctionType.Exp)
                nc.vector.reduce_sum(
                    out=ssum[:],
                    in_=ee[:].rearrange("p s (h d) -> p s h d", h=HEADS),
                    axis=mybir.AxisListType.X)
                # rpn = 1/(Sp*Sn) per block
                rpn = smallp.tile([C, HEADS], F32, tag="rpn")
                nc.vector.tensor_mul(out=rpn[:], in0=ssum[:, 0, :],
                                     in1=ssum[:, 1, :])
                nc.vector.reciprocal(out=rpn[:], in_=rpn[:])
                # rc[c] = rpn[c, head(c)] via masked reduce
                scrh = smallp.tile([C, HEADS], F32, tag="scrh")
                rc1 = smallp.tile([C, 1], F32, tag="rc1")
                nc.vector.tensor_mul(out=scrh[:], in0=rpn[:], in1=hmask[:])
                nc.vector.reduce_sum(out=rc1[:], in_=scrh[:],
                                     axis=mybir.AxisListType.X)
                pp = smallp.tile([C, C], F32, tag="pp")
                nc.vector.tensor_mul(out=pp[:], in0=ee[:, 0, :],
                                     in1=ee[:, 1, :])
                nc.vector.tensor_scalar_mul(out=pp[:], in0=pp[:],
                                            scalar1=rc1[:])
                e2 = smallp.tile([C, C], F32, tag="e2")
                nc.scalar.activation(out=e2[:], in_=pp[:],
                                     func=mybir.ActivationFunctionType.Exp)
                s2 = smallp.tile([C, HEADS], F32, tag="s2")
                nc.vector.reduce_sum(
                    out=s2[:], in_=e2[:].rearrange("p (h d) -> p h d", h=HEADS),
                    axis=mybir.AxisListType.X)
                nc.vector.reciprocal(out=s2[:], in_=s2[:])
                rc2 = smallp.tile([C, 1], F32, tag="rc2")
                nc.vector.tensor_mul(out=scrh[:], in0=s2[:], in1=hmask[:])
                nc.vector.reduce_sum(out=rc2[:], in_=scrh[:],
                                     axis=mybir.AxisListType.X)
                bd = smallp.tile([C, C], F32, tag="bd")
                nc.vector.tensor_scalar_mul(out=bd[:], in0=e2[:],
                                            scalar1=rc2[:])
                nc.vector.tensor_mul(out=bd[:], in0=bd[:], in1=bmask[:])
                mct_ps = psg.tile([C, C], F32, tag="g2")
                nc.tensor.matmul(mct_ps[:], lhsT=bd[:], rhs=wpt_sb[:],
                                 start=True, stop=True)
                nc.vector.tensor_copy(out=mct_sb[:, b, :], in_=mct_ps[:])

                # pass 2 for this b; 4-row bf16 output blocks, PSUM
                # evacuations alternating Act/DVE
                vview = v_sum[:, b, :].rearrange("p (r w) -> p r w", w=PITCH)
                for jj in range(RPC // 4):
                    osb = outp.tile([C, 4, PITCH], BF16)
                    for half in range(2):
                        j = 2 * jj + half
                        ops_ = psdw.tile([C, 2, 512], F32, tag="dwps")
                        for r2 in range(2):
                            nc.tensor.matmul(
                                ops_[:, r2, 0:PITCH], lhsT=mct_sb[:, b, :],
                                rhs=vview[:, 2 * j + r2, :],
                                start=True, stop=True)
                        dst = osb[:, 2 * half:2 * half + 2, :]
                        if half == 0:
                            nc.scalar.copy(out=dst, in_=ops_[:, :, 0:PITCH])
                        else:
                            nc.vector.tensor_copy(out=dst,
                                                  in_=ops_[:, :, 0:PITCH])
                    nc.sync.dma_start(out=y[b, :, 4 * jj:4 * jj + 4, :],
                                      in_=osb[:, :, 0:W])

            # ---------------- pass 1: conv + dw + grams + v_sum ----------
            sm_pending = None
            for b in range(N_B):
                x8_sb = {}
                xv_sb = {}
                fin_q = []   # deferred end-of-unit gram work
                for u in range(N_UNITS):
                    xi = [0, 1, 1, 2, 2][u]
                    qk = u in (0, 1, 3)
                    if qk:
                        if xi not in x8_sb:
                            t8 = x8pool.tile([C, 2, XLEN], F8, tag="x8")
                            for (a0, a1) in DMA_PIECES:
                                nc.sync.dma_start(
                                    out=t8[:, :, a0:a1],
                                    in_=x8src[xi][b][:, :, a0:a1])
                            x8_sb[xi] = t8
                        xt8 = x8_sb[xi]
                        slot = {0: 0, 1: 1, 3: 2}[u]
                    else:
                        if xi not in xv_sb:
                            tv = xvpool.tile([C, XLEN], BF16, tag="xv")
                            for (a0, a1) in DMA_PIECES:
                                nc.sync.dma_start(
                                    out=tv[:, a0:a1],
                                    in_=xvsrc[xi][b][:, a0:a1])
                            xv_sb[xi] = tv
                        xtv = xv_sb[xi]
                        wbase = 0 if u == 2 else 9

                    if u == 0:
                        ustore = qstore
                    elif u in (1, 3):
                        ustore = kstorep.tile([128, NTILES, C], F8, tag="kT")
                    else:
                        ustore = None

                    if u == 0:
                        g_self = psg.tile([C, C], F32, tag="g")
                        g_cross = None
                    elif u in (1, 3):
                        g_self = psg.tile([C, C], F32, tag="g")
                        g_cross = psg.tile([C, C], F32, tag="g2")

                    # software pipelining (q/k): PE stream per chunk j is
                    # [conv(j), transpose(j-1), grams(j-2)] so the PE never
                    # waits on the Act-engine PSUM evacuation of the chunk
                    # it just produced. End-of-unit grams are deferred into
                    # the next unit's first chunk.
                    def do_transp(dwsb):
                        tp = pstp.tile([128, 4, C], BF16)
                        for r2 in range(2):
                            for hf in range(2):
                                nc.tensor.transpose(
                                    tp[:, 2 * r2 + hf, :],
                                    dwsb[:, r2, 128 * hf:128 * hf + 128],
                                    identb[:],
                                )
                        return tp

                    def do_gram(i0, u=None, gs=None, gc=None, us=None):
                        for i in (i0, i0 + 2):
                            st = (i == 0)
                            sp = (i == 4 * N_CHUNK - 2)
                            if u == 0:
                                nc.tensor.matmul(
                                    gs[:],
                                    lhsT=qstore[:, i:i + 2, :],
                                    rhs=qstore[:, i:i + 2, :],
                                    start=st, stop=sp, perf_mode=DR,
                                    skip_group_check=True)
                            else:
                                nc.tensor.matmul(
                                    gc[:],
                                    lhsT=qstore[:, i:i + 2, :],
                                    rhs=us[:, i:i + 2, :],
                                    start=st, stop=sp, perf_mode=DR,
                                    skip_group_check=True)
                                nc.tensor.matmul(
                                    gs[:],
                                    lhsT=us[:, i:i + 2, :],
                                    rhs=us[:, i:i + 2, :],
                                    start=st, stop=sp, perf_mode=DR,
                                    skip_group_check=True)

                    def fin_unit(u=None, b=None, gs=None, gc=None, us=None):
                        do_gram(4 * (N_CHUNK - 2), u=u, gs=gs, gc=gc, us=us)
                        do_gram(4 * (N_CHUNK - 1), u=u, gs=gs, gc=gc, us=us)
                        if u == 0:
                            nc.vector.tensor_copy(out=gram_sb[:, b, 0, :],
                                                  in_=gs[:])
                        elif u == 1:
                            nc.vector.tensor_copy(out=gram_sb[:, b, 1, :],
                                                  in_=gc[:])
                            nc.vector.tensor_copy(out=gram_sb[:, b, 2, :],
                                                  in_=gs[:])
                        else:
                            nc.vector.tensor_copy(out=gram_sb[:, b, 3, :],
                                                  in_=gc[:])
                            nc.vector.tensor_copy(out=gram_sb[:, b, 4, :],
                                                  in_=gs[:])

                    dwsb_q = {}
                    for j in range(N_CHUNK):
                        dwps = psdw.tile([C, 2, 512], F32, tag="dwps")
                        for r2 in range(2):
                            r = 1 + 2 * j + r2
                            ro = rowoff(r)
                            if qk:
                                for p, (t1, t2, kind) in enumerate(PAIRS):
                                    a = ro + TAPS[t1]
                                    if kind == "A":
                                        rhs = xt8[:, :, a:a + PITCH]
                                    else:
                                        rhs = xt8[:, 0:1, a:a + 2 * PITCH]\
                                            .rearrange(
                                                "p one (two n) -> p (one two) n",
                                                two=2)
                                    nc.tensor.matmul(
                                        dwps[:, r2, 0:PITCH],
                                        lhsT=w8_sb[:, slot * 5 + p, :, :],
                                        rhs=rhs,
                                        start=(p == 0), stop=(p == 4),
                                        perf_mode=DR,
                                    )
                            else:
                                for t in range(9):
                                    a = ro + TAPS[t]
                                    nc.tensor.matmul(
                                        dwps[:, r2, 0:PITCH],
                                        lhsT=w3v_sb[:, wbase + t, :],
                                        rhs=xtv[:, a:a + PITCH],
                                        start=(t == 0), stop=(t == 8),
                                    )
                        if j == 0 and fin_q:
                            fin_q.pop(0)()
                        if u == 2 and j == 2 and sm_pending is not None:
                            # previous b's softmax + pass 2 slots into the
                            # middle of this v unit: no psg-arena conflict
                            # and the PE keeps conv work in flight while
                            # the softmax chain ping-pongs on DVE/Act
                            softmax_pass2(sm_pending)
                            sm_pending = None
                        if qk:
                            dwsb = dwsbp.tile([C, 2, PITCH], BF16)
                            nc.scalar.copy(out=dwsb[:], in_=dwps[:, :, 0:PITCH])
                            dwsb_q[j] = dwsb
                            if j >= 1:
                                tp = do_transp(dwsb_q.pop(j - 1))
                                nc.vector.tensor_copy(
                                    out=ustore[:, 4 * (j - 1):4 * (j - 1) + 4, :],
                                    in_=tp[:])
                            if j >= 2:
                                do_gram(4 * (j - 2), u=u, gs=g_self,
                                        gc=g_cross, us=ustore)
                        else:
                            vslice = v_sum[:, b, :].rearrange(
                                "p (r w) -> p r w", w=PITCH)[:, 2 * j:2 * j + 2, :]
                            if u == 2:
                                nc.vector.tensor_copy(
                                    out=vslice, in_=dwps[:, :, 0:PITCH])
                            else:
                                nc.vector.tensor_add(
                                    out=vslice, in0=dwps[:, :, 0:PITCH],
                                    in1=vslice)
                    if qk:
                        # drain transposes now; defer final grams + evac
                        tp = do_transp(dwsb_q.pop(N_CHUNK - 1))
                        nc.vector.tensor_copy(
                            out=ustore[:, 4 * (N_CHUNK - 1):4 * N_CHUNK, :],
                            in_=tp[:])
                        fin_q.append(lambda u=u, b=b, gs=g_self, gc=g_cross,
                                     us=ustore: fin_unit(u=u, b=b, gs=gs,
                                                         gc=gc, us=us))

                for f in fin_q:
                    f()
                fin_q = []
                if N_UNITS < 5 or SKIP_SM:
                    continue
                stats_ar(b)
                sm_pending = b

            if sm_pending is not None:
                softmax_pass2(sm_pending)

    nc.compile()
    return nc


def _flat_strip(x, c):
    """Guarded flat strip [B, C, XLEN] (fp32) for core c."""
    r0 = c * RPC - 1
    lo, hi = max(r0, 0), min(r0 + SROWS, H)
    body = np.zeros((B, C, SROWS, PITCH), np.float32)
    body[:, :, lo - r0:lo - r0 + hi - lo, 0:W] = x[:, :, lo:hi, :]
    out = np.zeros((B, C, XLEN), np.float32)
    out[:, :, LEAD:LEAD + SROWS * PITCH] = body.reshape(B, C, -1)
    return out


def _two_plane_f8(flat):
    """[B, C, XLEN] fp32 -> [B, C, 2, XLEN] fp8e4 (plane1 = flat shift +1)."""
    p1 = np.zeros_like(flat)
    p1[:, :, :XLEN - 1] = flat[:, :, 1:]
    return np.stack([flat, p1], axis=2).astype(NP_F8)


def _prep_inputs(inputs):
    """Build per-core in_maps from full inputs."""
    x_curr = np.asarray(inputs["x_curr"], np.float32)
    x_prev = np.asarray(inputs["x_prev"], np.float32)
    x_next = np.asarray(inputs["x_next"], np.float32)
    w_q = np.asarray(inputs["w_q"], np.float32)
    w_q_dw = np.asarray(inputs["w_q_dw"], np.float32)
    w_kv_prev = np.asarray(inputs["w_kv_prev"], np.float32)
    w_kv_dw_prev = np.asarray(inputs["w_kv_dw_prev"], np.float32)
    w_kv_next = np.asarray(inputs["w_kv_next"], np.float32)
    w_kv_dw_next = np.asarray(inputs["w_kv_dw_next"], np.float32)
    w_proj = np.asarray(inputs["w_proj"], np.float32)
    temperature = np.asarray(inputs["temperature"], np.float32)

    units = [
        (w_q, w_q_dw.reshape(C, 9)),
        (w_kv_prev[0:C], w_kv_dw_prev[0:C].reshape(C, 9)),
        (w_kv_prev[C:2 * C], w_kv_dw_prev[C:2 * C].reshape(C, 9)),
        (w_kv_next[0:C], w_kv_dw_next[0:C].reshape(C, 9)),
        (w_kv_next[C:2 * C], w_kv_dw_next[C:2 * C].reshape(C, 9)),
    ]
    # w3[u][c, t, o] = W1_u[o, c] * wdw_u[o, t]
    w3 = [np.einsum("oc,ot->cto", w1, wdw).astype(np.float32)
          for (w1, wdw) in units]

    # q/k units -> fp8 DoubleRow pair weights, rescaled into fp8 range
    # (scale cancels in the channel-attention L2 normalization)
    w8 = np.zeros((C, 15, 2, C), np.float32)
    for si, u in enumerate((0, 1, 3)):
        wu = w3[u]
        denom = np.sqrt((wu.astype(np.float64) ** 2).sum(axis=(0, 1)).mean())
        s = 16.0 / max(denom, 1e-30)
        for p, (t1, t2, _) in enumerate(PAIRS):
            w8[:, si * 5 + p, 0, :] = wu[:, t1, :] * s
            if t2 is not None:
                w8[:, si * 5 + p, 1, :] = wu[:, t2, :] * s
    w8 = w8.astype(NP_F8)

    # v units -> bf16 plain taps
    w3v = np.concatenate([w3[2], w3[4]], axis=1).astype(NP_BF16)

    wpt = np.ascontiguousarray(w_proj.T)
    tmpv = np.repeat(temperature.reshape(HEADS), CH).reshape(C, 1)
    tmpv = np.ascontiguousarray(tmpv, np.float32)
    hmk = np.zeros((C, HEADS), np.float32)
    for h in range(HEADS):
        hmk[h * CH:(h + 1) * CH, h] = 1.0
    bmk = np.zeros((C, C), np.float32)
    for h in range(HEADS):
        bmk[h * CH:(h + 1) * CH, h * CH:(h + 1) * CH] = 1.0

    in_maps = []
    for c in range(N_CORES):
        fc = _flat_strip(x_curr, c)
        fp = _flat_strip(x_prev, c)
        fn = _flat_strip(x_next, c)
        in_maps.append({
            "xc8": _two_plane_f8(fc),
            "xp8": _two_plane_f8(fp),
            "xn8": _two_plane_f8(fn),
            "xpv": fp.astype(NP_BF16),
            "xnv": fn.astype(NP_BF16),
            "w8": w8,
            "w3v": w3v,
            "wpt": wpt,
            "tmp": tmpv,
            "idn": np.eye(C, dtype=np.float32),
            "hmk": hmk,
            "bmk": bmk,
        })
    return in_maps


def kernel(**inputs):
    if "nc" not in _CACHE:
        _CACHE["nc"] = build_kernel()
    nc = _CACHE["nc"]
    in_maps = _prep_inputs(inputs)
    res = run_bass_kernel_spmd(nc, in_maps, core_ids=list(range(N_CORES)))
    out = np.empty((B, C, H, W), np.float32)
    for c in range(N_CORES):
        out[:, :, c * RPC:(c + 1) * RPC, :] = np.asarray(
            res.results[c]["y"]).astype(np.float32)
    return out


if __name__ == "__main__":
    rng = np.random.default_rng(0)
    inputs = {
        "x_curr": rng.standard_normal((B, C, H, W), np.float32),
        "x_prev": rng.standard_normal((B, C, H, W), np.float32),
        "x_next": rng.standard_normal((B, C, H, W), np.float32),
        "w_q": rng.standard_normal((C, C), np.float32) * 0.02,
        "w_q_dw": rng.standard_normal((C, 1, 3, 3), np.float32) * 0.02,
        "w_kv_prev": rng.standard_normal((2 * C, C), np.float32) * 0.02,
        "w_kv_dw_prev": rng.standard_normal((2 * C, 1, 3, 3), np.float32) * 0.02,
        "w_kv_next": rng.standard_normal((2 * C, C), np.float32) * 0.02,
        "w_kv_dw_next": rng.standard_normal((2 * C, 1, 3, 3), np.float32) * 0.02,
        "w_proj": rng.standard_normal((C, C), np.float32) * 0.02,
        "temperature": np.ones((HEADS, 1, 1), np.float32),
    }
    out = kernel(**inputs)
    print("out", out.shape, out.dtype, np.abs(out).max())


# revision 27
# speedup vs baseline: 1.7505x; 1.0297x over previous
"""Co-Attention kernel for Trainium2, 8-core SPMD.

Sharding: spatial (H rows) across 8 cores; 32 rows/core with 1-row halo.
Per-core pipeline (all fused, single launch):
  - host pre-builds guarded flat strips (258-pitch rows, zeros in guards):
    fp8e4 2-plane strips (plane1 = flat shift +1) for q/k, bf16 for v
  - conv1x1+dwconv3x3 folded: q/k units run 5 fp8 DoubleRow matmuls per row
    (2 taps per pass: 3 horizontal pairs via the +1 plane, 1 vertical pair
    via a stride-258 view, 1 zero-padded), v units run 9 bf16 matmuls
  - q/k weights are rescaled per unit into fp8 range; the L2 normalization
    of q/k makes the channel attention invariant to that scale
  - q/k: PE transpose -> fp8 [n,c] tiles -> Gram matrices via fp8 DoubleRow
    (2 spatial tiles per pass), accumulated over the core's spatial shard
  - v: v_prev+v_next accumulated into an SBUF-resident bf16 strip
  - AllReduce of the tiny Gram/norm stats across the 8 cores
  - on-chip double softmax (block-diagonal channel attention)
  - output = (w_proj @ blockdiag(attn_co)) @ v_sum, one matmul per chunk,
    DMA'd to DRAM directly from PSUM
"""

import sys

sys.path.insert(0, "/opt/trn_rl_repo")

import ml_dtypes
import numpy as np

import concourse.bacc as bacc
import concourse.bass as bass
import concourse.tile as tile
from concourse import mybir
from concourse.bass_utils import run_bass_kernel_spmd

# problem constants
B, C, H, W = 2, 96, 256, 256
HEADS = 4
CH = C // HEADS
N_CORES = 8
RPC = H // N_CORES          # rows per core (32)
SROWS = RPC + 2             # strip rows incl halo (34)
PITCH = W + 2               # guarded row pitch (258)
LEAD = 2                    # leading guard pad
XLEN = LEAD + SROWS * PITCH + 2  # strip flat length (8776)
NTILES = RPC * 2            # 128-wide transpose tiles per unit per b (64)
VLEN = RPC * W              # v_sum flat length per b, no guards (8192)

F32 = mybir.dt.float32
F32R = mybir.dt.float32r
BF16 = mybir.dt.bfloat16
F8 = mybir.dt.float8e4
DR = mybir.MatmulPerfMode.DoubleRow

NP_F8 = ml_dtypes.float8_e4m3
NP_BF16 = ml_dtypes.bfloat16

# tap offsets (cross-correlation, matching jax.lax.conv_general_dilated)
TAPS = [(ky - 1) * PITCH + (kx - 1) for ky in range(3) for kx in range(3)]
# DoubleRow tap pairs for q/k: ('A' = plane pair at +1, 'B' = vertical
# stride-258 pair on plane0, second tap None = zero-padded plane pair)
PAIRS = [(0, 1, "A"), (3, 4, "A"), (6, 7, "A"), (2, 5, "B"), (8, None, "A")]
# strip DMA split points (flat offsets; 3 pieces so conv can start after
# the first rows land)
_P1 = 2 + 5 * PITCH
_P2 = 2 + 16 * PITCH
DMA_PIECES = [(0, _P1), (_P1, _P2), (_P2, XLEN)]

_CACHE = {}


def rowoff(r):
    return LEAD + r * PITCH


def build_kernel():
    import os as _os
    N_UNITS = int(_os.environ.get("N_UNITS", "5"))
    N_B = int(_os.environ.get("N_B", str(B)))
    N_CHUNK = int(_os.environ.get("N_CHUNK", str(RPC // 2)))
    SKIP_SM = bool(_os.environ.get("SKIP_SM"))
    nc = bacc.Bacc("TRN2", target_bir_lowering=False, debug=False,
                   num_devices=N_CORES)

    xc8 = nc.declare_dram_parameter("xc8", [B, C, 2, XLEN], F8, isOutput=False)
    xp8 = nc.declare_dram_parameter("xp8", [B, C, 2, XLEN], F8, isOutput=False)
    xn8 = nc.declare_dram_parameter("xn8", [B, C, 2, XLEN], F8, isOutput=False)
    xpv = nc.declare_dram_parameter("xpv", [B, C, XLEN], BF16, isOutput=False)
    xnv = nc.declare_dram_parameter("xnv", [B, C, XLEN], BF16, isOutput=False)
    w8 = nc.declare_dram_parameter("w8", [C, 15, 2, C], F8, isOutput=False)
    w3v = nc.declare_dram_parameter("w3v", [C, 18, C], BF16, isOutput=False)
    wpt = nc.declare_dram_parameter("wpt", [C, C], F32, isOutput=False)
    tmp = nc.declare_dram_parameter("tmp", [C, 1], F32, isOutput=False)
    idn = nc.declare_dram_parameter("idn", [C, C], F32, isOutput=False)
    hmk = nc.declare_dram_parameter("hmk", [C, HEADS], F32, isOutput=False)
    bmk = nc.declare_dram_parameter("bmk", [C, C], F32, isOutput=False)
    y = nc.declare_dram_parameter("y", [B, C, RPC, W], BF16, isOutput=True)

    ar_in = [nc.dram_tensor(f"ar_in{i}", [C, 195], F32) for i in range(B)]
    ar_out = [nc.dram_tensor(f"ar_out{i}", [C, 195], F32, addr_space="Shared")
              for i in range(B)]

    x8src = {0: xc8, 1: xp8, 2: xn8}
    xvsrc = {1: xpv, 2: xnv}

    with tile.TileContext(nc) as tc:
        with (
            tc.tile_pool(name="singles", bufs=1) as singles,
            tc.tile_pool(name="x8pool", bufs=2) as x8pool,
            tc.tile_pool(name="xvpool", bufs=2) as xvpool,
            tc.tile_pool(name="dwsb", bufs=4) as dwsbp,
            tc.tile_pool(name="store", bufs=1) as storep,
            tc.tile_pool(name="kstore", bufs=2) as kstorep,
            tc.tile_pool(name="small", bufs=4) as smallp,
            tc.tile_pool(name="outp", bufs=4) as outp,
            tc.tile_pool(name="psdw", bufs=2, space="PSUM") as psdw,
            tc.tile_pool(name="pstp", bufs=2, space="PSUM") as pstp,
            tc.tile_pool(name="psg", bufs=1, space="PSUM") as psg,
        ):
            # ---- constants (first-needed first: w8 + the b=0 x_curr
            # strip gate the very first conv matmul) ----
            w8_sb = singles.tile([C, 15, 2, C], F8)
            nc.sync.dma_start(out=w8_sb[:], in_=w8[:, :, :, :])
            pre_x8 = x8pool.tile([C, 2, XLEN], F8, tag="x8")
            for (a0, a1) in DMA_PIECES:
                nc.sync.dma_start(out=pre_x8[:, :, a0:a1],
                                  in_=xc8[0][:, :, a0:a1])
            w3v_sb = singles.tile([C, 18, C], BF16)
            nc.sync.dma_start(out=w3v_sb[:], in_=w3v[:, :, :])
            wpt_sb = singles.tile([C, C], F32)
            nc.sync.dma_start(out=wpt_sb[:], in_=wpt[:, :])
            temp_sb = singles.tile([C, 1], F32)
            nc.sync.dma_start(out=temp_sb[:], in_=tmp[:, :])
            ident = singles.tile([C, C], F32)
            nc.sync.dma_start(out=ident[:], in_=idn[:, :])
            identb = singles.tile([C, C], BF16)
            nc.vector.tensor_copy(out=identb[:], in_=ident[:])
            hmask = singles.tile([C, HEADS], F32)
            nc.sync.dma_start(out=hmask[:], in_=hmk[:, :])
            bmask = singles.tile([C, C], F32)
            nc.sync.dma_start(out=bmask[:], in_=bmk[:, :])

            # persistent accumulators
            v_sum = singles.tile([C, B, VLEN], BF16)
            ar_sb = singles.tile([C, B, 195], F32)
            gram_sb = singles.tile([C, B, 5, C], F32)
            arr_sb = singles.tile([C, B, 195], F32)
            mct_sb = singles.tile([C, B, C], BF16)

            qstore = storep.tile([128, NTILES, C], F8)

            # ---------------- helper blocks ----------------
            def stats_ar(b):
                """Diag stats + per-b AllReduce (issued right after b's
                pass 1 so the collective overlaps the next b's compute)."""
                scr = smallp.tile([C, C], F32, tag="scr")
                for k, slot_ in enumerate((0, 2, 4)):
                    nc.vector.tensor_mul(out=scr[:],
                                         in0=gram_sb[:, b, slot_, :],
                                         in1=ident[:])
                    nc.vector.reduce_sum(out=ar_sb[:, b, 192 + k:193 + k],
                                         in_=scr[:],
                                         axis=mybir.AxisListType.X)
                nc.vector.tensor_copy(out=ar_sb[:, b, 0:96],
                                      in_=gram_sb[:, b, 1, :])
                nc.vector.tensor_copy(out=ar_sb[:, b, 96:192],
                                      in_=gram_sb[:, b, 3, :])
                import os as _os
                if _os.environ.get("SKIP_AR"):
                    nc.vector.tensor_copy(out=arr_sb[:, b, :],
                                          in_=ar_sb[:, b, :])
                else:
                    nc.sync.dma_start(out=ar_in[b][:, :], in_=ar_sb[:, b, :])
                    nc.gpsimd.collective_compute(
                        "AllReduce", mybir.AluOpType.add,
                        replica_groups=[list(range(N_CORES))],
                        ins=[ar_in[b][:, :]], outs=[ar_out[b][:, :]],
                    )
                    nc.sync.dma_start(out=arr_sb[:, b, :], in_=ar_out[b][:, :])

            p2_q = []

            def p2_group(b, jj):
                # 4 output rows: 2 matmuls of 512 cols (v_sum is guard-free
                # so 2 rows are contiguous), evacs alternating Act/DVE
                osb = outp.tile([C, 4, W], BF16)
                ops_ = psdw.tile([C, 2, 512], F32, tag="dwps")
                for h in range(2):
                    a = (4 * jj + 2 * h) * W
                    nc.tensor.matmul(
                        ops_[:, h, :], lhsT=mct_sb[:, b, :],
                        rhs=v_sum[:, b, a:a + 512],
                        start=True, stop=True)
                for h in range(2):
                    srcv = ops_[:, h, :].rearrange("p (r w) -> p r w", w=W)
                    dst = osb[:, 2 * h:2 * h + 2, :]
                    if h == 0:
                        nc.scalar.copy(out=dst, in_=srcv)
                    else:
                        nc.vector.tensor_copy(out=dst, in_=srcv)
                nc.sync.dma_start(out=y[b, :, 4 * jj:4 * jj + 4, :],
                                  in_=osb[:, :, :])

            def softmax_pass2(b):
                rinv = smallp.tile([C, 3], F32, tag="rinv")
                nc.scalar.activation(out=rinv[:], in_=arr_sb[:, b, 192:195],
                                     func=mybir.ActivationFunctionType.Sqrt)
                nc.vector.tensor_scalar_max(out=rinv[:], in0=rinv[:],
                                            scalar1=1e-12)
                nc.vector.reciprocal(out=rinv[:], in_=rinv[:])
                rqt = smallp.tile([C, 1], F32, tag="rqt")
                nc.vector.tensor_mul(out=rqt[:], in0=rinv[:, 0:1],
                                     in1=temp_sb[:])

                ee = smallp.tile([C, 2, C], F32, tag="ee")
                ssum = smallp.tile([C, 2, HEADS], F32, tag="ssum")
                logits = smallp.tile([C, 2, C], F32, tag="logits")
                nc.vector.tensor_scalar_mul(
                    out=logits[:],
                    in0=arr_sb[:, b, 0:192].rearrange("p (s c) -> p s c", s=2),
                    scalar1=rqt[:])
                # column scale via transpose sandwich, both branches at once:
                # Lt = L.T ; Lt *= rk (per-partition) ; L = Lt.T
                lt_ps = psg.tile([C, 2, C], F32, tag="g")
                for s in range(2):
                    nc.tensor.transpose(lt_ps[:, s, :], logits[:, s, :],
                                        ident[:])
                lts = smallp.tile([C, 2, C], F32, tag="lts")
                for s in range(2):
                    nc.vector.tensor_scalar_mul(out=lts[:, s, :],
                                                in0=lt_ps[:, s, :],
                                                scalar1=rinv[:, 1 + s:2 + s])
                lt2_ps = psg.tile([C, 2, C], F32, tag="g2")
                for s in range(2):
                    nc.tensor.transpose(lt2_ps[:, s, :], lts[:, s, :],
                                        ident[:])
                nc.scalar.activation(out=ee[:], in_=lt2_ps[:],
                                     func=mybir.ActivationFunctionType.Exp)
                nc.vector.reduce_sum(
                    out=ssum[:],
                    in_=ee[:].rearrange("p s (h d) -> p s h d", h=HEADS),
                    axis=mybir.AxisListType.X)
                # rpn = 1/(Sp*Sn) per block
                rpn = smallp.tile([C, HEADS], F32, tag="rpn")
                nc.vector.tensor_mul(out=rpn[:], in0=ssum[:, 0, :],
                                     in1=ssum[:, 1, :])
                nc.vector.reciprocal(out=rpn[:], in_=rpn[:])
                # rc[c] = rpn[c, head(c)] via masked reduce
                scrh = smallp.tile([C, HEADS], F32, tag="scrh")
                rc1 = smallp.tile([C, 1], F32, tag="rc1")
                nc.vector.tensor_mul(out=scrh[:], in0=rpn[:], in1=hmask[:])
                nc.vector.reduce_sum(out=rc1[:], in_=scrh[:],
                                     axis=mybir.AxisListType.X)
                pp = smallp.tile([C, C], F32, tag="pp")
                nc.vector.tensor_mul(out=pp[:], in0=ee[:, 0, :],
                                     in1=ee[:, 1, :])
                nc.vector.tensor_scalar_mul(out=pp[:], in0=pp[:],
                                            scalar1=rc1[:])
                e2 = smallp.tile([C, C], F32, tag="e2")
                nc.scalar.activation(out=e2[:], in_=pp[:],
                                     func=mybir.ActivationFunctionType.Exp)
                s2 = smallp.tile([C, HEADS], F32, tag="s2")
                nc.vector.reduce_sum(
                    out=s2[:], in_=e2[:].rearrange("p (h d) -> p h d", h=HEADS),
                    axis=mybir.AxisListType.X)
                nc.vector.reciprocal(out=s2[:], in_=s2[:])
                rc2 = smallp.tile([C, 1], F32, tag="rc2")
                nc.vector.tensor_mul(out=scrh[:], in0=s2[:], in1=hmask[:])
                nc.vector.reduce_sum(out=rc2[:], in_=scrh[:],
                                     axis=mybir.AxisListType.X)
                bd = smallp.tile([C, C], F32, tag="bd")
                nc.vector.tensor_scalar_mul(out=bd[:], in0=e2[:],
                                            scalar1=rc2[:])
                nc.vector.tensor_mul(out=bd[:], in0=bd[:], in1=bmask[:])
                mct_ps = psg.tile([C, C], F32, tag="g2")
                nc.tensor.matmul(mct_ps[:], lhsT=bd[:], rhs=wpt_sb[:],
                                 start=True, stop=True)
                nc.vector.tensor_copy(out=mct_sb[:, b, :], in_=mct_ps[:])

                # pass 2 groups are queued and dripped one-per-chunk
                # into the v-unit conv loops (see p2_q)
                for jj in range(RPC // 4):
                    p2_q.append(lambda b=b, jj=jj: p2_group(b, jj))

            # ---------------- pass 1: conv + dw + grams + v_sum ----------
            sm_pending = None
            for b in range(N_B):
                x8_sb = {0: pre_x8} if b == 0 else {}
                xv_sb = {}
                fin_q = []   # deferred end-of-unit gram work
                for u in range(N_UNITS):
                    xi = [0, 1, 1, 2, 2][u]
                    qk = u in (0, 1, 3)
                    if qk:
                        if xi not in x8_sb:
                            t8 = x8pool.tile([C, 2, XLEN], F8, tag="x8")
                            for (a0, a1) in DMA_PIECES:
                                nc.sync.dma_start(
                                    out=t8[:, :, a0:a1],
                                    in_=x8src[xi][b][:, :, a0:a1])
                            x8_sb[xi] = t8
                        xt8 = x8_sb[xi]
                        slot = {0: 0, 1: 1, 3: 2}[u]
                    else:
                        if xi not in xv_sb:
                            tv = xvpool.tile([C, XLEN], BF16, tag="xv")
                            for (a0, a1) in DMA_PIECES:
                                nc.sync.dma_start(
                                    out=tv[:, a0:a1],
                                    in_=xvsrc[xi][b][:, a0:a1])
                            xv_sb[xi] = tv
                        xtv = xv_sb[xi]
                        wbase = 0 if u == 2 else 9

                    if u == 0:
                        ustore = qstore
                    elif u in (1, 3):
                        ustore = kstorep.tile([128, NTILES, C], F8, tag="kT")
                    else:
                        ustore = None

                    if u == 0:
                        g_self = psg.tile([C, C], F32, tag="g")
                        g_cross = None
                    elif u in (1, 3):
                        g_self = psg.tile([C, C], F32, tag="g")
                        g_cross = psg.tile([C, C], F32, tag="g2")

                    # software pipelining (q/k): PE stream per chunk j is
                    # [conv(j), transpose(j-1), grams(j-2)] so the PE never
                    # waits on the Act-engine PSUM evacuation of the chunk
                    # it just produced. End-of-unit grams are deferred into
                    # the next unit's first chunk.
                    def do_transp(dwsb):
                        tp = pstp.tile([128, 4, C], BF16)
                        for r2 in range(2):
                            for hf in range(2):
                                nc.tensor.transpose(
                                    tp[:, 2 * r2 + hf, :],
                                    dwsb[:, r2, 128 * hf:128 * hf + 128],
                                    identb[:],
                                )
                        return tp

                    def do_gram(i0, u=None, gs=None, gc=None, us=None):
                        for i in (i0, i0 + 2):
                            st = (i == 0)
                            sp = (i == 4 * N_CHUNK - 2)
                            if u == 0:
                                nc.tensor.matmul(
                                    gs[:],
                                    lhsT=qstore[:, i:i + 2, :],
                                    rhs=qstore[:, i:i + 2, :],
                                    start=st, stop=sp, perf_mode=DR,
                                    skip_group_check=True)
                            else:
                                nc.tensor.matmul(
                                    gc[:],
                                    lhsT=qstore[:, i:i + 2, :],
                                    rhs=us[:, i:i + 2, :],
                                    start=st, stop=sp, perf_mode=DR,
                                    skip_group_check=True)
                                nc.tensor.matmul(
                                    gs[:],
                                    lhsT=us[:, i:i + 2, :],
                                    rhs=us[:, i:i + 2, :],
                                    start=st, stop=sp, perf_mode=DR,
                                    skip_group_check=True)

                    def fin_unit(u=None, b=None, gs=None, gc=None, us=None):
                        do_gram(4 * (N_CHUNK - 2), u=u, gs=gs, gc=gc, us=us)
                        do_gram(4 * (N_CHUNK - 1), u=u, gs=gs, gc=gc, us=us)
                        if u == 0:
                            nc.vector.tensor_copy(out=gram_sb[:, b, 0, :],
                                                  in_=gs[:])
                        elif u == 1:
                            nc.vector.tensor_copy(out=gram_sb[:, b, 1, :],
                                                  in_=gc[:])
                            nc.vector.tensor_copy(out=gram_sb[:, b, 2, :],
                                                  in_=gs[:])
                        else:
                            nc.vector.tensor_copy(out=gram_sb[:, b, 3, :],
                                                  in_=gc[:])
                            nc.vector.tensor_copy(out=gram_sb[:, b, 4, :],
                                                  in_=gs[:])

                    dwsb_q = {}
                    for j in range(N_CHUNK):
                        dwps = psdw.tile([C, 2, 512], F32, tag="dwps")
                        for r2 in range(2):
                            r = 1 + 2 * j + r2
                            ro = rowoff(r)
                            if qk:
                                for p, (t1, t2, kind) in enumerate(PAIRS):
                                    a = ro + TAPS[t1]
                                    if kind == "A":
                                        rhs = xt8[:, :, a:a + PITCH]
                                    else:
                                        rhs = xt8[:, 0:1, a:a + 2 * PITCH]\
                                            .rearrange(
                                                "p one (two n) -> p (one two) n",
                                                two=2)
                                    nc.tensor.matmul(
                                        dwps[:, r2, 0:PITCH],
                                        lhsT=w8_sb[:, slot * 5 + p, :, :],
                                        rhs=rhs,
                                        start=(p == 0), stop=(p == 4),
                                        perf_mode=DR,
                                    )
                            else:
                                for t in range(9):
                                    a = ro + TAPS[t]
                                    nc.tensor.matmul(
                                        dwps[:, r2, 0:PITCH],
                                        lhsT=w3v_sb[:, wbase + t, :],
                                        rhs=xtv[:, a:a + PITCH],
                                        start=(t == 0), stop=(t == 8),
                                    )
                        if j == 1 and fin_q:
                            fin_q.pop(0)()
                        if (not qk) and j >= 3 and p2_q:
                            p2_q.pop(0)()
                        if u == 2 and j == 2 and sm_pending is not None:
                            # previous b's softmax + pass 2 slots into the
                            # middle of this v unit: no psg-arena conflict
                            # and the PE keeps conv work in flight while
                            # the softmax chain ping-pongs on DVE/Act
                            softmax_pass2(sm_pending)
                            sm_pending = None
                        if qk:
                            dwsb = dwsbp.tile([C, 2, PITCH], BF16)
                            nc.scalar.copy(out=dwsb[:], in_=dwps[:, :, 0:PITCH])
                            dwsb_q[j] = dwsb
                            if j >= 1:
                                tp = do_transp(dwsb_q.pop(j - 1))
                                nc.vector.tensor_copy(
                                    out=ustore[:, 4 * (j - 1):4 * (j - 1) + 4, :],
                                    in_=tp[:])
                            if j >= 2:
                                do_gram(4 * (j - 2), u=u, gs=g_self,
                                        gc=g_cross, us=ustore)
                        else:
                            vslice = v_sum[:, b, :].rearrange(
                                "p (r w) -> p r w", w=W)[:, 2 * j:2 * j + 2, :]
                            if u == 2:
                                nc.vector.tensor_copy(
                                    out=vslice, in_=dwps[:, :, 0:W])
                            else:
                                nc.vector.tensor_add(
                                    out=vslice, in0=dwps[:, :, 0:W],
                                    in1=vslice)
                    if qk:
                        # drain transposes now; defer final grams + evac
                        tp = do_transp(dwsb_q.pop(N_CHUNK - 1))
                        nc.vector.tensor_copy(
                            out=ustore[:, 4 * (N_CHUNK - 1):4 * N_CHUNK, :],
                            in_=tp[:])
                        fin_q.append(lambda u=u, b=b, gs=g_self, gc=g_cross,
                                     us=ustore: fin_unit(u=u, b=b, gs=gs,
                                                         gc=gc, us=us))

                for f in fin_q:
                    f()
                fin_q = []
                if N_UNITS < 5 or SKIP_SM:
                    continue
                stats_ar(b)
                sm_pending = b

            if sm_pending is not None:
                softmax_pass2(sm_pending)
            while p2_q:
                p2_q.pop(0)()

    nc.compile()
    return nc


def _flat_strip(x, c):
    """Guarded flat strip [B, C, XLEN] (fp32) for core c."""
    r0 = c * RPC - 1
    lo, hi = max(r0, 0), min(r0 + SROWS, H)
    body = np.zeros((B, C, SROWS, PITCH), np.float32)
    body[:, :, lo - r0:lo - r0 + hi - lo, 0:W] = x[:, :, lo:hi, :]
    out = np.zeros((B, C, XLEN), np.float32)
    out[:, :, LEAD:LEAD + SROWS * PITCH] = body.reshape(B, C, -1)
    return out


def _two_plane_f8(flat):
    """[B, C, XLEN] fp32 -> [B, C, 2, XLEN] fp8e4 (plane1 = flat shift +1)."""
    p1 = np.zeros_like(flat)
    p1[:, :, :XLEN - 1] = flat[:, :, 1:]
    return np.stack([flat, p1], axis=2).astype(NP_F8)


def _prep_inputs(inputs):
    """Build per-core in_maps from full inputs."""
    x_curr = np.asarray(inputs["x_curr"], np.float32)
    x_prev = np.asarray(inputs["x_prev"], np.float32)
    x_next = np.asarray(inputs["x_next"], np.float32)
    w_q = np.asarray(inputs["w_q"], np.float32)
    w_q_dw = np.asarray(inputs["w_q_dw"], np.float32)
    w_kv_prev = np.asarray(inputs["w_kv_prev"], np.float32)
    w_kv_dw_prev = np.asarray(inputs["w_kv_dw_prev"], np.float32)
    w_kv_next = np.asarray(inputs["w_kv_next"], np.float32)
    w_kv_dw_next = np.asarray(inputs["w_kv_dw_next"], np.float32)
    w_proj = np.asarray(inputs["w_proj"], np.float32)
    temperature = np.asarray(inputs["temperature"], np.float32)

    units = [
        (w_q, w_q_dw.reshape(C, 9)),
        (w_kv_prev[0:C], w_kv_dw_prev[0:C].reshape(C, 9)),
        (w_kv_prev[C:2 * C], w_kv_dw_prev[C:2 * C].reshape(C, 9)),
        (w_kv_next[0:C], w_kv_dw_next[0:C].reshape(C, 9)),
        (w_kv_next[C:2 * C], w_kv_dw_next[C:2 * C].reshape(C, 9)),
    ]
    # w3[u][c, t, o] = W1_u[o, c] * wdw_u[o, t]
    w3 = [np.einsum("oc,ot->cto", w1, wdw).astype(np.float32)
          for (w1, wdw) in units]

    # q/k units -> fp8 DoubleRow pair weights, rescaled into fp8 range
    # (scale cancels in the channel-attention L2 normalization)
    w8 = np.zeros((C, 15, 2, C), np.float32)
    for si, u in enumerate((0, 1, 3)):
        wu = w3[u]
        denom = np.sqrt((wu.astype(np.float64) ** 2).sum(axis=(0, 1)).mean())
        s = 16.0 / max(denom, 1e-30)
        for p, (t1, t2, _) in enumerate(PAIRS):
            w8[:, si * 5 + p, 0, :] = wu[:, t1, :] * s
            if t2 is not None:
                w8[:, si * 5 + p, 1, :] = wu[:, t2, :] * s
    w8 = w8.astype(NP_F8)

    # v units -> bf16 plain taps
    w3v = np.concatenate([w3[2], w3[4]], axis=1).astype(NP_BF16)

    wpt = np.ascontiguousarray(w_proj.T)
    tmpv = np.repeat(temperature.reshape(HEADS), CH).reshape(C, 1)
    tmpv = np.ascontiguousarray(tmpv, np.float32)
    hmk = np.zeros((C, HEADS), np.float32)
    for h in range(HEADS):
        hmk[h * CH:(h + 1) * CH, h] = 1.0
    bmk = np.zeros((C, C), np.float32)
    for h in range(HEADS):
        bmk[h * CH:(h + 1) * CH, h * CH:(h + 1) * CH] = 1.0

    in_maps = []
    for c in range(N_CORES):
        fc = _flat_strip(x_curr, c)
        fp = _flat_strip(x_prev, c)
        fn = _flat_strip(x_next, c)
        in_maps.append({
            "xc8": _two_plane_f8(fc),
            "xp8": _two_plane_f8(fp),
            "xn8": _two_plane_f8(fn),
            "xpv": fp.astype(NP_BF16),
            "xnv": fn.astype(NP_BF16),
            "w8": w8,
            "w3v": w3v,
            "wpt": wpt,
            "tmp": tmpv,
            "idn": np.eye(C, dtype=np.float32),
            "hmk": hmk,
            "bmk": bmk,
        })
    return in_maps


def kernel(**inputs):
    if "nc" not in _CACHE:
        _CACHE["nc"] = build_kernel()
    nc = _CACHE["nc"]
    in_maps = _prep_inputs(inputs)
    res = run_bass_kernel_spmd(nc, in_maps, core_ids=list(range(N_CORES)))
    out = np.empty((B, C, H, W), np.float32)
    for c in range(N_CORES):
        out[:, :, c * RPC:(c + 1) * RPC, :] = np.asarray(
            res.results[c]["y"]).astype(np.float32)
    return out


if __name__ == "__main__":
    rng = np.random.default_rng(0)
    inputs = {
        "x_curr": rng.standard_normal((B, C, H, W), np.float32),
        "x_prev": rng.standard_normal((B, C, H, W), np.float32),
        "x_next": rng.standard_normal((B, C, H, W), np.float32),
        "w_q": rng.standard_normal((C, C), np.float32) * 0.02,
        "w_q_dw": rng.standard_normal((C, 1, 3, 3), np.float32) * 0.02,
        "w_kv_prev": rng.standard_normal((2 * C, C), np.float32) * 0.02,
        "w_kv_dw_prev": rng.standard_normal((2 * C, 1, 3, 3), np.float32) * 0.02,
        "w_kv_next": rng.standard_normal((2 * C, C), np.float32) * 0.02,
        "w_kv_dw_next": rng.standard_normal((2 * C, 1, 3, 3), np.float32) * 0.02,
        "w_proj": rng.standard_normal((C, C), np.float32) * 0.02,
        "temperature": np.ones((HEADS, 1, 1), np.float32),
    }
    out = kernel(**inputs)
    print("out", out.shape, out.dtype, np.abs(out).max())
